# revision 15
# baseline (speedup 1.0000x reference)
"""Trainium2 Bass kernel for nn_AgentModel (negotiation agent forward pass).

Self-contained: takes FULL inputs, shards batch across 8 NeuronCores (pure
data parallel), runs one fp32 Bass/Tile program per core, gathers outputs.

Numerical strategy: all jax.random draws in the reference are input-
independent (uniform/normal/gumbel with fixed keys+shapes), so they are
precomputed on host with jax-CPU and fed to the device. All sampling
decisions (categorical argmax, greedy argmax, bernoulli compare) are made
from exact fp32 arithmetic on shifted logits -- the coarse ACT exp/ln LUTs
only touch output values (nodes/entropy) whose tolerances are loose.
"""
import numpy as np

E = 100
EPS = 1e-8
CORR = 0.1
MAX_LEN = 6
NUM_ITEMS = 3
NCORES = 8
BTOT = 32768
BP = BTOT // NCORES          # 4096 per core
NT = BP // 128               # 32 batch tiles of 128
NH = NT // 2                 # 16 tiles per half
BH = BP // 2                 # 2048 per half
NQT = NT // 4                # 8 tiles per quarter
BQ = BP // 4                 # 1024 per quarter
CH = 512                     # matmul N chunk (fp32 limit / PSUM bank)
NCH = BP // CH               # 8 chunks
BIG = 1e9

_CACHE = {}


# ---------------------------------------------------------------- sync fixup
def _split_excess_sync(nc, wait_limit=1, update_limit=1):
    """walrus rejects >1 sync wait/update per instruction in this toolchain;
    split extras onto adjacent same-engine NoOps (engines execute in order)."""
    import concourse.mybir as mybir
    n = 0
    for f in nc.m.functions:
        for bb in f.blocks:
            new_insts = []
            for ins in bb.instructions:
                si = ins.sync_info
                waits = list(si.on_wait) if si and si.on_wait else []
                updates = list(si.on_update) if si and si.on_update else []
                pre, post = [], []
                while len(waits) > wait_limit:
                    chunk, waits = waits[:wait_limit], waits[wait_limit:]
                    n += 1
                    pre.append(mybir.InstNoOp(
                        name=f"{ins.name}-ws{len(pre)}", engine=ins.engine,
                        ins=[], outs=[],
                        sync_info=mybir.SyncInfo(on_wait=chunk, on_update=[])))
                while len(updates) > update_limit:
                    chunk, updates = updates[:update_limit], updates[update_limit:]
                    n += 1
                    post.append(mybir.InstNoOp(
                        name=f"{ins.name}-us{len(post)}", engine=ins.engine,
                        ins=[], outs=[],
                        sync_info=mybir.SyncInfo(on_wait=[], on_update=chunk)))
                if si is not None:
                    si.on_wait = waits
                    si.on_update = updates
                new_insts.extend(pre)
                new_insts.append(ins)
                new_insts.extend(post)
            if len(new_insts) != len(bb.instructions):
                bb.instructions[:] = new_insts
    return n


# ---------------------------------------------------------------- program
def _build_program():
    import concourse.bass as bass
    import concourse.tile as tile
    from concourse import mybir
    from contextlib import ExitStack

    F32 = mybir.dt.float32
    I32 = mybir.dt.int32
    U8 = mybir.dt.uint8
    AF = mybir.ActivationFunctionType
    OP = mybir.AluOpType
    AX = mybir.AxisListType

    nc = bass.Bass("TRN2", target_bir_lowering=False, debug=False,
                   num_devices=NCORES)

    def dram_in(name, shape, dt=F32):
        return nc.dram_tensor(name, shape, dt, kind="ExternalInput")

    def dram_out(name, shape, dt=F32):
        return nc.dram_tensor(name, shape, dt, kind="ExternalOutput")

    tok_d = dram_in("tok", [15, BP])
    ubern_d = dram_in("ubern", [128, NT])
    rnp_d = dram_in("rnp", [128, MAX_LEN, NT, 10])
    gumu_d = dram_in("gumu", [128, MAX_LEN, NT, 10])
    gump_d = dram_in("gump", [128, NUM_ITEMS, NT, 6])
    BF16 = mybir.dt.bfloat16
    xw_d = {k: [dram_in(f"xw{p}_{k}", [v, 4 * E], BF16) for p in "hl"]
            for k, v in [("ctx", 6), ("m", 10), ("pr", 6), ("up", 10)]}
    hw_d = {k: [dram_in(f"hw{p}_{k}", [E, 4 * E], BF16) for p in "hl"]
            for k in ("ctx", "m", "pr", "up")}
    VOC = {"ctx": 6, "m": 10, "pr": 6, "up": 10}
    wc_d = {k: [dram_in(f"wc{p}_{k}", [E + VOC[k], 4 * E], BF16) for p in "hl"]
            for k in ("ctx", "m", "pr", "up")}
    b4_d = {k: dram_in(f"b4_{k}", [E, 4]) for k in ("ctx", "m", "pr", "up")}
    cwT_d = dram_in("cwT", [300, E])
    combb_d = dram_in("combb", [E, 1])
    headW_d = dram_in("headW", [E, 19])
    headb_d = dram_in("headb", [128, 19])
    uhW_d = dram_in("uhW", [E, 10])
    uhb_d = dram_in("uhb", [128, 10])

    nodes_o = dram_out("nodes_o", [BP, 10])
    aterm_o = dram_out("aterm_o", [BP], U8)
    utt_o = dram_out("utt_o", [BP, MAX_LEN], I32)
    prop_o = dram_out("prop_o", [BP, NUM_ITEMS], I32)
    scal_o = dram_out("scal_o", [1, 6])

    def row_bcast_ap(handle, row, nrows, ncols):
        """DRAM [R, C] row -> AP broadcasting that row across nrows partitions."""
        return bass.AP(tensor=handle, offset=row * ncols,
                       ap=[[0, nrows], [1, ncols]])

    def bc(ap3, n):
        """[128, X] AP -> [128, X, n] via trailing 0-stride broadcast."""
        return ap3.to_broadcast(list(ap3.shape) + [n])

    def bcmid(ap2, n):
        """[128, X] AP -> [128, n, X] via middle 0-stride broadcast."""
        return bass.AP(tensor=ap2.tensor, offset=ap2.offset,
                       ap=[list(ap2.ap[0]), [0, n], list(ap2.ap[1])])

    ctx = ExitStack()
    with tile.TileContext(nc) as tc, ctx:
        wsb = ctx.enter_context(tc.tile_pool(name="wsb", bufs=1))
        res = ctx.enter_context(tc.tile_pool(name="res", bufs=1))
        ohp = ctx.enter_context(tc.tile_pool(name="ohp", bufs=1))
        hp = ctx.enter_context(tc.tile_pool(name="hp", bufs=2))
        cp = ctx.enter_context(tc.tile_pool(name="cp", bufs=2))
        go = ctx.enter_context(tc.tile_pool(name="go", bufs=6))
        hd = ctx.enter_context(tc.tile_pool(name="hd", bufs=18))
        rgp = ctx.enter_context(tc.tile_pool(name="rgp", bufs=2))
        hd19 = ctx.enter_context(tc.tile_pool(name="hd19", bufs=2))
        psp = ctx.enter_context(tc.tile_pool(name="psp", bufs=8, space="PSUM"))
        drp = ctx.enter_context(tc.tile_pool(name="drp", bufs=4, space="DRAM"))

        # ---- load weights ----
        def wload(dram, shape, tag):
            t = wsb.tile(shape, F32, tag=tag, name=tag)
            nc.sync.dma_start(t[:], dram.ap())
            return t

        def wload2(pair, shape, tag):
            out = []
            for p, dram in zip("hl", pair):
                t = wsb.tile(shape, BF16, tag=f"{tag}{p}", name=f"{tag}{p}")
                nc.sync.dma_start(t[:], dram.ap())
                out.append(t)
            return out

        xw = {k: wload2(d, [d[0].shape[0], 4 * E], f"xw{k}") for k, d in xw_d.items()}
        hw = {k: wload2(d, [E, 4 * E], f"hw{k}") for k, d in hw_d.items()}
        wc = {k: wload2(d, [E + VOC[k], 4 * E], f"wc{k}") for k, d in wc_d.items()}
        b4 = {k: wload(d, [E, 4], f"b4{k}") for k, d in b4_d.items()}
        cwT = [wsb.tile([E, E], F32, tag=f"cwT{i}", name=f"cwT{i}") for i in range(3)]
        for i in range(3):
            nc.sync.dma_start(cwT[i][:], cwT_d.ap()[i * E:(i + 1) * E, :])
        combb = wload(combb_d, [E, 1], "combb")
        headW = wload(headW_d, [E, 19], "headW")
        headb = wload(headb_d, [128, 19], "headb")
        uhW = wload(uhW_d, [E, 10], "uhW")
        uhb = wload(uhb_d, [128, 10], "uhb")

        # ---- resident data ----
        ubern = res.tile([128, NT], F32, tag="ubern", name="ubern")
        nc.sync.dma_start(ubern[:], ubern_d.ap())
        gump = res.tile([128, NUM_ITEMS, NT, 6], F32, tag="gump", name="gump")
        nc.sync.dma_start(gump[:], gump_d.ap())

        # iota over class dim [128, NH, 10] and per-partition vocab index [10,1]
        iotaI = res.tile([128, NH, 10], I32, tag="iotaI", name="iotaI")
        nc.gpsimd.iota(iotaI[:], pattern=[[0, NH], [1, 10]], base=0,
                       channel_multiplier=0)
        iotaC = res.tile([128, NH, 10], F32, tag="iotaC", name="iotaC")
        nc.vector.tensor_copy(iotaC[:], iotaI[:])
        vidxI = res.tile([10, 1], I32, tag="vidxI", name="vidxI")
        nc.gpsimd.iota(vidxI[:], pattern=[[0, 1]], base=0, channel_multiplier=1)
        vidx = res.tile([10, 1], F32, tag="vidx", name="vidx")
        nc.vector.tensor_copy(vidx[:], vidxI[:])

        ones = res.tile([128, 1], F32, tag="ones", name="ones")
        nc.vector.memset(ones[:], 1.0)

        # accumulators: rows 0..5 = S_term, S_utt, S_prop, m_term, m_utt, m_prop
        acc = res.tile([128, 6, NT], F32, tag="acc", name="acc")
        nc.vector.memset(acc[:], 0.0)
        nodes_a = res.tile([128, NT, 10], F32, tag="nodes_a", name="nodes_a")
        utt_a = res.tile([128, NT, MAX_LEN], I32, tag="utt_a", name="utt_a")
        prop_a = res.tile([128, NT, NUM_ITEMS], I32, tag="prop_a", name="prop_a")
        aterm_a = res.tile([128, NT], U8, tag="aterm_a", name="aterm_a")

        comb_acc = res.tile([E, BP], F32, tag="comb_acc", name="comb_acc")
        h_t = res.tile([E, BP], F32, tag="h_t", name="h_t")
        hcp = ctx.enter_context(tc.tile_pool(name="hcp", bufs=2))
        ohq_p = ctx.enter_context(tc.tile_pool(name="ohq_p", bufs=2))
        hlp = ctx.enter_context(tc.tile_pool(name="hlp", bufs=2))

        # ---- one-hot build ----
        def build_onehot(row, vocab):
            # step-0 onehot at base partition 0 (standalone x-side matmul)
            oh = ohp.tile([10, BP], BF16, tag="oh", name="oh")
            nc.gpsimd.dma_start(oh[:vocab, :], row_bcast_ap(tok_d, row, vocab, BP))
            nc.gpsimd.tensor_scalar(oh[:vocab, :], oh[:vocab, :], vidx[:vocab, :],
                                    None, op0=OP.is_equal)
            return oh

        def build_onehot_hc(row, vocab, hc):
            # compute-engine partition ranges must be 32-aligned, so build the
            # onehot at base partition 0 and DMA it into rows E..E+vocab
            oh = ohp.tile([10, BP], BF16, tag="oh", name="oh")
            nc.gpsimd.dma_start(oh[:vocab, :], row_bcast_ap(tok_d, row, vocab, BP))
            nc.gpsimd.tensor_scalar(oh[:vocab, :], oh[:vocab, :],
                                    vidx[:vocab, :], None, op0=OP.is_equal)
            nc.sync.dma_start(hc[E:E + vocab, :], oh[:vocab, :])

        # ---- LSTM step (feature-land), cols = chunk range ----
        def lstm_chunks(xw_t, vocab, wc_t, hw_t, b4_t, oh, h_prev, c_prev,
                        h_new, c_new, chunks, split_to=None):
            # h_prev: None (h=0, oh = standalone onehot tile) or (hc, hl) where
            # hc = [hh; onehot] combined bf16 tile; c_prev None => c=0
            xwh, xwl = xw_t
            wch, wcl = wc_t
            hwh, hwl = hw_t
            KC = E + vocab
            ch_list = list(chunks)
            for k0 in range(0, len(ch_list), 2):
                grp = ch_list[k0:k0 + 2]
                pst = {(g, k): psp.tile([E, CH], F32, tag="ps", name="ps")
                       for g in range(4) for k in grp}
                if h_prev is None:
                    passes = [(xwh[:vocab, :], oh[:vocab, :], True),
                              (xwl[:vocab, :], oh[:vocab, :], False)]
                else:
                    hc, hl = h_prev
                    passes = [(wch[:KC, :], hc[:KC, :], True),
                              (wcl[:KC, :], hc[:KC, :], False),
                              (hwh[:, :], hl[:, :], False),
                              (hwl[:, :], hl[:, :], False)]
                np_ = len(passes)
                # weight-reuse: same stationary operand streams both chunks
                for g in range(4):
                    gs = slice(g * E, (g + 1) * E)
                    for pi, (w, rhs_t, st) in enumerate(passes):
                        wv = w[:, gs]
                        for k in grp:
                            cols = slice(k * CH, (k + 1) * CH)
                            nc.tensor.matmul(pst[(g, k)][:], wv,
                                             rhs_t[:, cols],
                                             start=st, stop=pi == np_ - 1,
                                             skip_group_check=True)
                for k in grp:
                    cols = slice(k * CH, (k + 1) * CH)
                    ig = go.tile([E, CH], F32, tag="go", name="go")
                    nc.scalar.activation(ig[:], pst[(0, k)][:], AF.Sigmoid,
                                         bias=b4_t[:, 0:1])
                    gg = go.tile([E, CH], F32, tag="go", name="go")
                    nc.scalar.activation(gg[:], pst[(2, k)][:], AF.Tanh,
                                         bias=b4_t[:, 2:3])
                    og = go.tile([E, CH], F32, tag="go", name="go")
                    nc.scalar.activation(og[:], pst[(3, k)][:], AF.Sigmoid,
                                         bias=b4_t[:, 3:4])
                    if c_prev is None:
                        nc.vector.tensor_mul(c_new[:, cols], ig[:], gg[:])
                    else:
                        fg = go.tile([E, CH], F32, tag="go", name="go")
                        nc.scalar.activation(fg[:], pst[(1, k)][:], AF.Sigmoid,
                                             bias=b4_t[:, 1:2])
                        t1 = go.tile([E, CH], F32, tag="go", name="go")
                        nc.vector.tensor_mul(t1[:], ig[:], gg[:])
                        t2 = go.tile([E, CH], F32, tag="go", name="go")
                        nc.gpsimd.tensor_mul(t2[:], fg[:], c_prev[:, cols])
                        nc.vector.tensor_add(c_new[:, cols], t1[:], t2[:])
                    th = go.tile([E, CH], F32, tag="go", name="go")
                    nc.scalar.activation(th[:], c_new[:, cols], AF.Tanh)
                    nc.vector.tensor_mul(h_new[:, cols], og[:], th[:])
                    if split_to is not None:
                        shc, shl = split_to
                        nc.gpsimd.tensor_copy(shc[:E, cols], h_new[:, cols])
                        nc.vector.tensor_tensor(shl[:, cols], h_new[:, cols],
                                                shc[:E, cols], op=OP.subtract)

        # ---- encoder ----
        def encoder(key, vocab, row0, steps):
            hsp = c_prev = None
            oh = build_onehot(row0, vocab)
            for s in range(steps):
                h_new = hp.tile([E, BP], F32, tag="h", name="h")
                c_new = cp.tile([E, BP], F32, tag="c", name="c")
                split_to = None
                if s + 1 < steps:
                    hc_next = hcp.tile([112, BP], BF16, tag="hc", name="hc")
                    build_onehot_hc(row0 + s + 1, vocab, hc_next)
                    split_to = (hc_next,
                                hlp.tile([E, BP], BF16, tag="hl", name="hl"))
                lstm_chunks(xw[key], vocab, wc[key], hw[key], b4[key], oh, hsp,
                            c_prev, h_new, c_new, range(NCH), split_to)
                hsp, c_prev = split_to, c_new
                oh = None
            return h_new

        def comb_add(enc_idx, h_enc):
            for k in range(NCH):
                cols = slice(k * CH, (k + 1) * CH)
                ps = psp.tile([128, CH], F32, tag="ps", name="ps")
                nc.tensor.matmul(ps[:E, :], cwT[enc_idx][:], h_enc[:, cols],
                                 start=True, stop=True)
                if enc_idx == 0:
                    nc.vector.tensor_copy(comb_acc[:, cols], ps[:E, :])
                else:
                    nc.vector.tensor_add(comb_acc[:, cols], comb_acc[:, cols],
                                         ps[:E, :])

        c_h = encoder("ctx", 6, 0, 6)
        comb_add(0, c_h)
        m_h = encoder("m", 10, 6, 6)
        comb_add(1, m_h)
        p_h = encoder("pr", 6, 12, 3)
        comb_add(2, p_h)
        nc.scalar.activation(h_t[:], comb_acc[:], AF.Relu, bias=combb[:, 0:1])

        # ---- shared softmax/sample in head-land ----
        def softmax_sample(l2, C, nt, gum_ap, hsl, nodes_col, tok_slice,
                           ent_row, match_row):
            """l2: [128, nt, C] logits AP. Returns sampled index tile [128, nt]."""
            smax = hd.tile([128, nt], F32, tag="hs", name="hs")
            nc.vector.tensor_reduce(smax[:], l2, axis=AX.X, op=OP.max)
            sh = hd.tile([128, nt, 10], F32, tag="hd", name="hd")[:, :, :C]
            nc.vector.tensor_tensor(sh, l2, bc(smax[:], C), op=OP.subtract)
            ex = hd.tile([128, nt, 10], F32, tag="hd", name="hd")[:, :, :C]
            nc.scalar.activation(ex, sh, AF.Exp)
            S = hd.tile([128, nt], F32, tag="hs", name="hs")
            nc.vector.tensor_reduce(S[:], ex, axis=AX.X, op=OP.add)
            rec = hd.tile([128, nt], F32, tag="hs", name="hs")
            nc.vector.reciprocal(rec[:], S[:])
            probs = hd.tile([128, nt, 10], F32, tag="hd", name="hd")[:, :, :C]
            nc.vector.tensor_tensor(probs, ex, bc(rec[:], C), op=OP.mult)
            # entropy contribution sum_c (p+EPS)ln(p+EPS)
            q = hd.tile([128, nt, 10], F32, tag="hd", name="hd")[:, :, :C]
            nc.vector.tensor_single_scalar(q, probs, EPS, op=OP.add)
            lnq = hd.tile([128, nt, 10], F32, tag="hd", name="hd")[:, :, :C]
            nc.scalar.activation(lnq, q, AF.Ln)
            ql = hd.tile([128, nt, 10], F32, tag="hd", name="hd")[:, :, :C]
            nc.vector.tensor_mul(ql, q, lnq)
            ctr = hd.tile([128, nt], F32, tag="hs", name="hs")
            nc.vector.tensor_reduce(ctr[:], ql, axis=AX.X, op=OP.add)
            nc.vector.tensor_add(acc[:, ent_row, hsl], acc[:, ent_row, hsl],
                                 ctr[:])
            # categorical sample: argmax(sh + gumbel) (lnS shift cancels)
            v = hd.tile([128, nt, 10], F32, tag="hd", name="hd")[:, :, :C]
            nc.vector.tensor_tensor(v, sh, gum_ap, op=OP.add)
            vmax = hd.tile([128, nt], F32, tag="hs", name="hs")
            nc.vector.tensor_reduce(vmax[:], v, axis=AX.X, op=OP.max)
            veq = hd.tile([128, nt, 10], F32, tag="hd", name="hd")[:, :, :C]
            nc.vector.tensor_tensor(veq, v, bc(vmax[:], C), op=OP.is_equal)
            msk = hd.tile([128, nt, 10], F32, tag="hd", name="hd")[:, :, :C]
            nc.vector.tensor_mul(msk, veq, iotaC[:, :nt, :C])
            pen = hd.tile([128, nt, 10], F32, tag="hd", name="hd")[:, :, :C]
            nc.vector.tensor_scalar(pen, veq, -BIG, BIG, op0=OP.mult, op1=OP.add)
            nc.vector.tensor_add(msk, msk, pen)
            af = hd.tile([128, nt], F32, tag="hs", name="hs")
            nc.vector.tensor_reduce(af[:], msk, axis=AX.X, op=OP.min)
            nc.vector.tensor_copy(tok_slice, af[:])        # cast f32 -> i32
            # chosen prob -> nodes; greedy match via shifted-logit == 0
            aeq = hd.tile([128, nt, 10], F32, tag="hd", name="hd")[:, :, :C]
            nc.vector.tensor_tensor(aeq, iotaC[:, :nt, :C], bc(af[:], C),
                                    op=OP.is_equal)
            pa = hd.tile([128, nt, 10], F32, tag="hd", name="hd")[:, :, :C]
            nc.vector.tensor_mul(pa, probs, aeq)
            pa1 = hd.tile([128, nt], F32, tag="hs", name="hs")
            nc.vector.tensor_reduce(pa1[:], pa, axis=AX.X, op=OP.add)
            nc.scalar.activation(nodes_a[:, hsl, nodes_col], pa1[:], AF.Ln)
            ssel = hd.tile([128, nt, 10], F32, tag="hd", name="hd")[:, :, :C]
            nc.vector.tensor_mul(ssel, sh, aeq)
            ss1 = hd.tile([128, nt], F32, tag="hs", name="hs")
            nc.vector.tensor_reduce(ss1[:], ssel, axis=AX.X, op=OP.add)
            mt = hd.tile([128, nt], F32, tag="hs", name="hs")
            nc.vector.tensor_single_scalar(mt[:], ss1[:], 0.0, op=OP.is_equal)
            nc.vector.tensor_add(acc[:, match_row, hsl], acc[:, match_row, hsl],
                                 mt[:])
            return af

        # ---- term + proposal heads ----
        for half in range(2):
            hsl = slice(half * NH, (half + 1) * NH)
            ps19 = psp.tile([128, NH, 19], F32, tag="ps", name="ps")
            for t16 in range(NH):
                t = half * NH + t16
                nc.tensor.matmul(ps19[:, t16, :], h_t[:, t * 128:(t + 1) * 128],
                                 headW[:], start=True, stop=True)
            lg19 = hd19.tile([128, NH, 19], F32, tag="hd19", name="hd19")
            nc.vector.tensor_tensor(lg19[:], ps19[:], bcmid(headb[:], NH),
                                    op=OP.add)
            # term policy
            tp = hd.tile([128, NH], F32, tag="hs", name="hs")
            nc.scalar.activation(tp[:], lg19[:, :, 0], AF.Sigmoid)
            au = hd.tile([128, NH], F32, tag="hs", name="hs")
            nc.vector.tensor_tensor(au[:], ubern[:, hsl], tp[:], op=OP.is_lt)
            nc.vector.tensor_copy(aterm_a[:, hsl], au[:])   # cast -> u8
            rg = hd.tile([128, NH], F32, tag="hs", name="hs")
            nc.vector.tensor_single_scalar(rg[:], tp[:], 0.5, op=OP.is_ge)
            mt = hd.tile([128, NH], F32, tag="hs", name="hs")
            nc.vector.tensor_tensor(mt[:], rg[:], au[:], op=OP.is_equal)
            nc.vector.tensor_add(acc[:, 3, hsl], acc[:, 3, hsl], mt[:])
            # g = a*p + (1-a)*(1-p), exact fp32 mirror of the reference
            aup = hd.tile([128, NH], F32, tag="hs", name="hs")
            nc.vector.tensor_mul(aup[:], au[:], tp[:])
            nau = hd.tile([128, NH], F32, tag="hs", name="hs")
            nc.vector.tensor_scalar(nau[:], au[:], -1.0, 1.0, op0=OP.mult,
                                    op1=OP.add)
            ntp = hd.tile([128, NH], F32, tag="hs", name="hs")
            nc.vector.tensor_scalar(ntp[:], tp[:], -1.0, 1.0, op0=OP.mult,
                                    op1=OP.add)
            t2 = hd.tile([128, NH], F32, tag="hs", name="hs")
            nc.vector.tensor_mul(t2[:], nau[:], ntp[:])
            gt = hd.tile([128, NH], F32, tag="hs", name="hs")
            nc.vector.tensor_add(gt[:], aup[:], t2[:])
            nc.scalar.activation(nodes_a[:, hsl, 0], gt[:], AF.Ln)
            q = hd.tile([128, NH], F32, tag="hs", name="hs")
            nc.vector.tensor_single_scalar(q[:], tp[:], EPS, op=OP.add)
            lnq = hd.tile([128, NH], F32, tag="hs", name="hs")
            nc.scalar.activation(lnq[:], q[:], AF.Ln)
            ql = hd.tile([128, NH], F32, tag="hs", name="hs")
            nc.vector.tensor_mul(ql[:], q[:], lnq[:])
            nc.vector.tensor_add(acc[:, 0, hsl], acc[:, 0, hsl], ql[:])
            # proposal heads
            for i in range(NUM_ITEMS):
                softmax_sample(lg19[:, :, 1 + 6 * i:7 + 6 * i], 6, NH,
                               gump[:, i, hsl, :], hsl, 7 + i,
                               prop_a[:, hsl, i], 2, 5)

        # ---- utterance decode (autoregressive, pipelined in quarters) ----
        hc0 = hcp.tile([112, BP], BF16, tag="hc", name="hc")
        oh0 = ohp.tile([10, BP], BF16, tag="oh", name="oh")
        nc.vector.memset(oh0[:], 0.0)
        nc.vector.memset(oh0[0:1, :], 1.0)
        nc.sync.dma_start(hc0[E:E + 10, :], oh0[:])
        h_tl = hlp.tile([E, BP], BF16, tag="hl", name="hl")
        nc.gpsimd.tensor_copy(hc0[:E, :], h_t[:])
        nc.vector.tensor_tensor(h_tl[:], h_t[:], hc0[:E, :], op=OP.subtract)
        hsp = (hc0, h_tl)
        c_cur = None
        for s in range(MAX_LEN):
            rnp_s = rgp.tile([128, NT, 10], F32, tag="rnp", name="rnp")
            nc.sync.dma_start(rnp_s[:], rnp_d.ap()[:, s, :, :])
            gumu_s = rgp.tile([128, NT, 10], F32, tag="gumu", name="gumu")
            nc.sync.dma_start(gumu_s[:], gumu_d.ap()[:, s, :, :])
            h_new = hp.tile([E, BP], F32, tag="h", name="h")
            c_new = cp.tile([E, BP], F32, tag="c", name="c")
            split_to = None
            if s + 1 < MAX_LEN:
                split_to = (hcp.tile([112, BP], BF16, tag="hc", name="hc"),
                            hlp.tile([E, BP], BF16, tag="hl", name="hl"))
            for q in range(4):
                lstm_chunks(xw["up"], 10, wc["up"], hw["up"], b4["up"], None,
                            hsp, c_cur, h_new, c_new,
                            range(q * 2, q * 2 + 2), split_to)
                qsl = slice(q * NQT, (q + 1) * NQT)
                ps10 = psp.tile([128, NQT, 10], F32, tag="ps", name="ps")
                for tq in range(NQT):
                    t = q * NQT + tq
                    nc.tensor.matmul(ps10[:, tq, :],
                                     h_new[:, t * 128:(t + 1) * 128],
                                     uhW[:], start=True, stop=True)
                lg0 = hd.tile([128, NQT, 10], F32, tag="hd", name="hd")
                nc.vector.tensor_tensor(lg0[:], ps10[:], bcmid(uhb[:], NQT),
                                        op=OP.add)
                # noise blend: l2 = 0.9*l + 0.1*(lmin + (lmax-lmin)*rn)
                rmn = hd.tile([128, NQT], F32, tag="hs", name="hs")
                nc.vector.tensor_reduce(rmn[:], lg0[:], axis=AX.X, op=OP.min)
                rmx = hd.tile([128, NQT], F32, tag="hs", name="hs")
                nc.vector.tensor_reduce(rmx[:], lg0[:], axis=AX.X, op=OP.max)
                d = hd.tile([128, NQT], F32, tag="hs", name="hs")
                nc.vector.tensor_tensor(d[:], rmx[:], rmn[:], op=OP.subtract)
                n1 = hd.tile([128, NQT, 10], F32, tag="hd", name="hd")
                nc.vector.tensor_tensor(n1[:], rnp_s[:, qsl, :], bc(d[:], 10),
                                        op=OP.mult)
                noise = hd.tile([128, NQT, 10], F32, tag="hd", name="hd")
                nc.vector.tensor_tensor(noise[:], n1[:], bc(rmn[:], 10),
                                        op=OP.add)
                ns = hd.tile([128, NQT, 10], F32, tag="hd", name="hd")
                nc.vector.tensor_single_scalar(ns[:], noise[:], CORR, op=OP.mult)
                l2 = hd.tile([128, NQT, 10], F32, tag="hd", name="hd")
                nc.vector.scalar_tensor_tensor(l2[:], lg0[:], 1.0 - CORR, ns[:],
                                               op0=OP.mult, op1=OP.add)
                af = softmax_sample(l2[:], 10, NQT, gumu_s[:, qsl, :], qsl,
                                    1 + s, utt_a[:, qsl, s], 1, 4)
                if s + 1 < MAX_LEN:
                    hc_next = split_to[0]
                    rt = drp.tile([BQ], F32, tag="rt", name="rt")
                    rbase = rt[:]
                    nc.sync.dma_start(
                        bass.AP(tensor=rbase.tensor, offset=rbase.offset,
                                ap=[[1, 128], [128, NQT]]), af[:])
                    cols = slice(q * BQ, (q + 1) * BQ)
                    ohq = ohq_p.tile([10, BQ], BF16, tag="ohq", name="ohq")
                    nc.gpsimd.dma_start(
                        ohq[:],
                        bass.AP(tensor=rbase.tensor, offset=rbase.offset,
                                ap=[[0, 10], [1, BQ]]))
                    nc.gpsimd.tensor_scalar(ohq[:], ohq[:], vidx[:], None,
                                            op0=OP.is_equal)
                    nc.sync.dma_start(hc_next[E:E + 10, cols], ohq[:])
            c_cur = c_new
            hsp = split_to

        # ---- final scalars ----
        red6 = hd.tile([128, 6], F32, tag="red6", name="red6")
        nc.vector.tensor_reduce(red6[:], acc[:], axis=AX.X, op=OP.add)
        ps_s = psp.tile([1, 6], F32, tag="ps", name="ps")
        nc.tensor.matmul(ps_s[:], ones[:], red6[:], start=True, stop=True)
        ssb = hd.tile([1, 6], F32, tag="ssb", name="ssb")
        nc.vector.tensor_copy(ssb[:], ps_s[:])
        nc.sync.dma_start(scal_o.ap(), ssb[:])

        # ---- output DMAs (head-land -> [BP, k] row-major) ----
        def out_ap(handle, k):
            return bass.AP(tensor=handle, offset=0,
                           ap=[[k, 128], [128 * k, NT], [1, k]])

        nc.sync.dma_start(out_ap(nodes_o, 10), nodes_a[:])
        nc.sync.dma_start(out_ap(utt_o, MAX_LEN), utt_a[:])
        nc.sync.dma_start(out_ap(prop_o, NUM_ITEMS), prop_a[:])
        nc.sync.dma_start(
            bass.AP(tensor=aterm_o, offset=0, ap=[[1, 128], [128, NT]]),
            aterm_a[:])

    _split_excess_sync(nc)
    return nc


# ---------------------------------------------------------------- host prep
def _rng_draws():
    if "rng" in _CACHE:
        return _CACHE["rng"]
    import jax
    import jax.numpy as jnp
    cpu = jax.devices('cpu')[0]
    with jax.default_device(cpu):
        rng = jax.random.key(1234)
        u = np.asarray(jax.random.uniform(jax.random.fold_in(rng, 0),
                                          (BTOT, 1), jnp.float32))
        raws = [np.asarray(jax.random.normal(jax.random.fold_in(rng, 100 + i),
                                             (BTOT, 10), jnp.float32))
                for i in range(MAX_LEN)]
        gu = [np.asarray(jax.random.gumbel(jax.random.fold_in(rng, 200 + i),
                                           (BTOT, 10), jnp.float32))
              for i in range(MAX_LEN)]
        gp = [np.asarray(jax.random.gumbel(jax.random.fold_in(rng, 300 + i),
                                           (BTOT, 6), jnp.float32))
              for i in range(NUM_ITEMS)]
    rn = []
    for r in raws:
        nmin = r.min(axis=1, keepdims=True)
        nmax = r.max(axis=1, keepdims=True)
        rn.append(((r - nmin) / (nmax - nmin)).astype(np.float32))
    _CACHE["rng"] = (u, rn, gu, gp)
    return _CACHE["rng"]


def _hl(x):
    """[BP, k] (or [BP]) -> head-land [128, NT, k] / [128, NT]."""
    if x.ndim == 1:
        return np.ascontiguousarray(x.reshape(NT, 128).T)
    return np.ascontiguousarray(
        x.reshape(NT, 128, x.shape[1]).transpose(1, 0, 2))


def _bf16_split(x):
    import ml_dtypes
    bf16 = ml_dtypes.bfloat16
    xh = x.astype(bf16)
    xl = (x - xh.astype(np.float32)).astype(bf16)
    return np.ascontiguousarray(xh), np.ascontiguousarray(xl)


def _prep_in_maps(pool, utility, m_prev, prev_proposal, params):
    f32 = np.float32
    P = {k: np.asarray(v, f32) for k, v in params.items()}
    toks = np.stack([np.asarray(t, np.int64)[:, j].astype(f32)
                     for t, jr in [(pool, range(3)), (utility, range(3)),
                                   (m_prev, range(6)), (prev_proposal, range(3))]
                     for j in jr])  # [15, BTOT]

    wmap = {
        "b4_ctx": np.ascontiguousarray((P["ctx_bih"] + P["ctx_bhh"]).reshape(4, E).T),
        "b4_m": np.ascontiguousarray((P["utt_bih"] + P["utt_bhh"]).reshape(4, E).T),
        "b4_pr": np.ascontiguousarray((P["prop_bih"] + P["prop_bhh"]).reshape(4, E).T),
        "b4_up": np.ascontiguousarray((P["up_bih"] + P["up_bhh"]).reshape(4, E).T),
        "cwT": np.ascontiguousarray(P["comb_W"].T),
        "combb": P["comb_b"].reshape(E, 1),
        "headW": np.concatenate([P["term_W"].T] +
                                [P["pp_W"][i].T for i in range(NUM_ITEMS)], axis=1),
        "headb": np.broadcast_to(
            np.concatenate([P["term_b"]] + [P["pp_b"][i] for i in range(NUM_ITEMS)]),
            (128, 19)).copy(),
        "uhW": np.ascontiguousarray(P["up_h1_W"].T),
        "uhb": np.broadcast_to(P["up_h1_b"], (128, 10)).copy(),
    }
    wmap = {k: np.ascontiguousarray(v, dtype=f32) for k, v in wmap.items()}
    for key, xwm, hwm in [
            ("ctx", P["emb_ctx"] @ P["ctx_Wih"].T, P["ctx_Whh"].T),
            ("m", P["emb_utt"] @ P["utt_Wih"].T, P["utt_Whh"].T),
            ("pr", P["emb_ctx"] @ P["prop_Wih"].T, P["prop_Whh"].T),
            ("up", P["up_emb"] @ P["up_Wih"].T, P["up_Whh"].T)]:
        xh, xl = _bf16_split(np.ascontiguousarray(xwm, dtype=f32))
        hh, hl = _bf16_split(np.ascontiguousarray(hwm, dtype=f32))
        wmap[f"xwh_{key}"], wmap[f"xwl_{key}"] = xh, xl
        wmap[f"hwh_{key}"], wmap[f"hwl_{key}"] = hh, hl
        wmap[f"wch_{key}"] = np.concatenate([hh, xh], axis=0)
        wmap[f"wcl_{key}"] = np.concatenate([hl, xl], axis=0)

    u, rn, gu, gp = _rng_draws()
    in_maps = []
    for c in range(NCORES):
        sl = slice(c * BP, (c + 1) * BP)
        m = dict(wmap)
        m["tok"] = np.ascontiguousarray(toks[:, sl])
        m["ubern"] = _hl(u[sl, 0])
        m["rnp"] = np.ascontiguousarray(
            np.stack([_hl(rn[i][sl]) for i in range(MAX_LEN)], axis=1))
        m["gumu"] = np.ascontiguousarray(
            np.stack([_hl(gu[i][sl]) for i in range(MAX_LEN)], axis=1))
        m["gump"] = np.ascontiguousarray(
            np.stack([_hl(gp[i][sl]) for i in range(NUM_ITEMS)], axis=1))
        in_maps.append(m)
    return in_maps


def _get_nc():
    if "nc" not in _CACHE:
        _CACHE["nc"] = _build_program()
    return _CACHE["nc"]


def _run(in_maps, trace=False, trace_kwargs=None):
    from concourse.bass_utils import run_bass_kernel_spmd
    nc = _get_nc()
    kw = {}
    if trace:
        kw["trace"] = True
        if trace_kwargs:
            kw["trace_kwargs"] = trace_kwargs
    return run_bass_kernel_spmd(nc, in_maps, core_ids=list(range(NCORES)), **kw)


def _assemble(results):
    nodes = np.concatenate([r["nodes_o"] for r in results], axis=0)
    a_term = np.concatenate([r["aterm_o"] for r in results])[:, None]
    utterance = np.concatenate([r["utt_o"] for r in results], axis=0)
    proposal = np.concatenate([r["prop_o"] for r in results], axis=0)
    scal = np.stack([r["scal_o"][0] for r in results]).astype(np.float64)
    s_term, s_utt, s_prop, m_term, m_utt, m_prop = scal.sum(axis=0)
    entropy_loss = np.float32(0.05 * s_term + 0.001 * s_utt + 0.05 * s_prop)
    return (nodes.astype(np.float32), a_term.astype(np.uint8),
            utterance.astype(np.int32), proposal.astype(np.int32),
            entropy_loss, np.int32(round(m_term)), np.int32(round(m_utt)),
            MAX_LEN * BTOT, np.int32(round(m_prop)), NUM_ITEMS * BTOT)


def kernel(pool, utility, m_prev, prev_proposal, params):
    in_maps = _prep_in_maps(pool, utility, m_prev, prev_proposal, params)
    res = _run(in_maps, trace=False)
    return _assemble(res.results)


# revision 16
# speedup vs baseline: 1.4721x; 1.4721x over previous
"""Trainium2 Bass kernel for nn_AgentModel (negotiation agent forward pass).

Self-contained: takes FULL inputs, shards batch across 8 NeuronCores (pure
data parallel), runs one fp32 Bass/Tile program per core, gathers outputs.

Numerical strategy: all jax.random draws in the reference are input-
independent (uniform/normal/gumbel with fixed keys+shapes), so they are
precomputed on host with jax-CPU and fed to the device. All sampling
decisions (categorical argmax, greedy argmax, bernoulli compare) are made
from exact fp32 arithmetic on shifted logits -- the coarse ACT exp/ln LUTs
only touch output values (nodes/entropy) whose tolerances are loose.
"""
import numpy as np

E = 100
EPS = 1e-8
CORR = 0.1
MAX_LEN = 6
NUM_ITEMS = 3
NCORES = 8
BTOT = 32768
BP = BTOT // NCORES          # 4096 per core
NT = BP // 128               # 32 batch tiles of 128
NH = NT // 2                 # 16 tiles per half
BH = BP // 2                 # 2048 per half
NQT = NT // 4                # 8 tiles per quarter
BQ = BP // 4                 # 1024 per quarter
CH = 512                     # matmul N chunk (fp32 limit / PSUM bank)
NCH = BP // CH               # 8 chunks
BIG = 1e9

_CACHE = {}


# ---------------------------------------------------------------- sync fixup
def _split_excess_sync(nc, wait_limit=1, update_limit=1):
    """walrus rejects >1 sync wait/update per instruction in this toolchain;
    split extras onto adjacent same-engine NoOps (engines execute in order)."""
    import concourse.mybir as mybir
    n = 0
    for f in nc.m.functions:
        for bb in f.blocks:
            new_insts = []
            for ins in bb.instructions:
                si = ins.sync_info
                waits = list(si.on_wait) if si and si.on_wait else []
                updates = list(si.on_update) if si and si.on_update else []
                pre, post = [], []
                while len(waits) > wait_limit:
                    chunk, waits = waits[:wait_limit], waits[wait_limit:]
                    n += 1
                    pre.append(mybir.InstNoOp(
                        name=f"{ins.name}-ws{len(pre)}", engine=ins.engine,
                        ins=[], outs=[],
                        sync_info=mybir.SyncInfo(on_wait=chunk, on_update=[])))
                while len(updates) > update_limit:
                    chunk, updates = updates[:update_limit], updates[update_limit:]
                    n += 1
                    post.append(mybir.InstNoOp(
                        name=f"{ins.name}-us{len(post)}", engine=ins.engine,
                        ins=[], outs=[],
                        sync_info=mybir.SyncInfo(on_wait=[], on_update=chunk)))
                if si is not None:
                    si.on_wait = waits
                    si.on_update = updates
                new_insts.extend(pre)
                new_insts.append(ins)
                new_insts.extend(post)
            if len(new_insts) != len(bb.instructions):
                bb.instructions[:] = new_insts
    return n


# ---------------------------------------------------------------- program
def _build_program():
    import concourse.bass as bass
    import concourse.tile as tile
    from concourse import mybir
    from contextlib import ExitStack

    F32 = mybir.dt.float32
    I32 = mybir.dt.int32
    U8 = mybir.dt.uint8
    AF = mybir.ActivationFunctionType
    OP = mybir.AluOpType
    AX = mybir.AxisListType

    nc = bass.Bass("TRN2", target_bir_lowering=False, debug=False,
                   num_devices=NCORES)

    def dram_in(name, shape, dt=F32):
        return nc.dram_tensor(name, shape, dt, kind="ExternalInput")

    def dram_out(name, shape, dt=F32):
        return nc.dram_tensor(name, shape, dt, kind="ExternalOutput")

    tok_d = dram_in("tok", [15, BP])
    ubern_d = dram_in("ubern", [128, NT])
    rnp_d = dram_in("rnp", [128, MAX_LEN, NT, 10])
    gumu_d = dram_in("gumu", [128, MAX_LEN, NT, 10])
    gump_d = dram_in("gump", [128, NUM_ITEMS, NT, 6])
    BF16 = mybir.dt.bfloat16
    xw_d = {k: [dram_in(f"xw{p}_{k}", [v, 4 * E], BF16) for p in "hl"]
            for k, v in [("ctx", 6), ("m", 10), ("pr", 6), ("up", 10)]}
    hw_d = {k: [dram_in(f"hw{p}_{k}", [E, 4 * E], BF16) for p in "hl"]
            for k in ("ctx", "m", "pr", "up")}
    VOC = {"ctx": 6, "m": 10, "pr": 6, "up": 10}
    wc_d = {k: [dram_in(f"wc{p}_{k}", [E + VOC[k], 4 * E], BF16) for p in "hl"]
            for k in ("ctx", "m", "pr", "up")}
    b4_d = {k: dram_in(f"b4_{k}", [E, 4]) for k in ("ctx", "m", "pr", "up")}
    cwT_d = dram_in("cwT", [300, E])
    combb_d = dram_in("combb", [E, 1])
    headW_d = dram_in("headW", [E, 19])
    headb_d = dram_in("headb", [128, 19])
    uhW_d = dram_in("uhW", [E, 10])
    uhb_d = dram_in("uhb", [128, 10])

    nodes_o = dram_out("nodes_o", [BP, 10])
    aterm_o = dram_out("aterm_o", [BP], U8)
    utt_o = dram_out("utt_o", [BP, MAX_LEN], I32)
    prop_o = dram_out("prop_o", [BP, NUM_ITEMS], I32)
    scal_o = dram_out("scal_o", [1, 6])

    def row_bcast_ap(handle, row, nrows, ncols):
        """DRAM [R, C] row -> AP broadcasting that row across nrows partitions."""
        return bass.AP(tensor=handle, offset=row * ncols,
                       ap=[[0, nrows], [1, ncols]])

    def bc(ap3, n):
        """[128, X] AP -> [128, X, n] via trailing 0-stride broadcast."""
        return ap3.to_broadcast(list(ap3.shape) + [n])

    def bcmid(ap2, n):
        """[128, X] AP -> [128, n, X] via middle 0-stride broadcast."""
        return bass.AP(tensor=ap2.tensor, offset=ap2.offset,
                       ap=[list(ap2.ap[0]), [0, n], list(ap2.ap[1])])

    ctx = ExitStack()
    with tile.TileContext(nc) as tc, ctx:
        wsb = ctx.enter_context(tc.tile_pool(name="wsb", bufs=1))
        res = ctx.enter_context(tc.tile_pool(name="res", bufs=1))
        ohp = ctx.enter_context(tc.tile_pool(name="ohp", bufs=1))
        hp = ctx.enter_context(tc.tile_pool(name="hp", bufs=2))
        cp = ctx.enter_context(tc.tile_pool(name="cp", bufs=2))
        go = ctx.enter_context(tc.tile_pool(name="go", bufs=6))
        hd = ctx.enter_context(tc.tile_pool(name="hd", bufs=18))
        rgp = ctx.enter_context(tc.tile_pool(name="rgp", bufs=2))
        hd19 = ctx.enter_context(tc.tile_pool(name="hd19", bufs=2))
        psp = ctx.enter_context(tc.tile_pool(name="psp", bufs=8, space="PSUM"))
        drp = ctx.enter_context(tc.tile_pool(name="drp", bufs=4, space="DRAM"))

        # ---- load weights ----
        def wload(dram, shape, tag):
            t = wsb.tile(shape, F32, tag=tag, name=tag)
            nc.sync.dma_start(t[:], dram.ap())
            return t

        def wload2(pair, shape, tag):
            out = []
            for p, dram in zip("hl", pair):
                t = wsb.tile(shape, BF16, tag=f"{tag}{p}", name=f"{tag}{p}")
                nc.sync.dma_start(t[:], dram.ap())
                out.append(t)
            return out

        xw = {k: wload2(d, [d[0].shape[0], 4 * E], f"xw{k}") for k, d in xw_d.items()}
        hw = {k: wload2(d, [E, 4 * E], f"hw{k}") for k, d in hw_d.items()}
        wc = {k: wload2(d, [E + VOC[k], 4 * E], f"wc{k}") for k, d in wc_d.items()}
        b4 = {k: wload(d, [E, 4], f"b4{k}") for k, d in b4_d.items()}
        cwT = [wsb.tile([E, E], F32, tag=f"cwT{i}", name=f"cwT{i}") for i in range(3)]
        for i in range(3):
            nc.sync.dma_start(cwT[i][:], cwT_d.ap()[i * E:(i + 1) * E, :])
        combb = wload(combb_d, [E, 1], "combb")
        headW = wload(headW_d, [E, 19], "headW")
        headb = wload(headb_d, [128, 19], "headb")
        uhW = wload(uhW_d, [E, 10], "uhW")
        uhb = wload(uhb_d, [128, 10], "uhb")

        # ---- resident data ----
        ubern = res.tile([128, NT], F32, tag="ubern", name="ubern")
        nc.sync.dma_start(ubern[:], ubern_d.ap())
        gump = res.tile([128, NUM_ITEMS, NT, 6], F32, tag="gump", name="gump")
        nc.sync.dma_start(gump[:], gump_d.ap())

        # iota over class dim [128, NH, 10] and per-partition vocab index [10,1]
        iotaI = res.tile([128, NH, 10], I32, tag="iotaI", name="iotaI")
        nc.gpsimd.iota(iotaI[:], pattern=[[0, NH], [1, 10]], base=0,
                       channel_multiplier=0)
        iotaC = res.tile([128, NH, 10], F32, tag="iotaC", name="iotaC")
        nc.vector.tensor_copy(iotaC[:], iotaI[:])
        vidxI = res.tile([10, 1], I32, tag="vidxI", name="vidxI")
        nc.gpsimd.iota(vidxI[:], pattern=[[0, 1]], base=0, channel_multiplier=1)
        vidx = res.tile([10, 1], F32, tag="vidx", name="vidx")
        nc.vector.tensor_copy(vidx[:], vidxI[:])

        ones = res.tile([128, 1], F32, tag="ones", name="ones")
        nc.vector.memset(ones[:], 1.0)

        # accumulators: rows 0..5 = S_term, S_utt, S_prop, m_term, m_utt, m_prop
        acc = res.tile([128, 6, NT], F32, tag="acc", name="acc")
        nc.vector.memset(acc[:], 0.0)
        nodes_a = res.tile([128, NT, 10], F32, tag="nodes_a", name="nodes_a")
        utt_a = res.tile([128, NT, MAX_LEN], I32, tag="utt_a", name="utt_a")
        prop_a = res.tile([128, NT, NUM_ITEMS], I32, tag="prop_a", name="prop_a")
        aterm_a = res.tile([128, NT], U8, tag="aterm_a", name="aterm_a")

        comb_acc = res.tile([E, BP], F32, tag="comb_acc", name="comb_acc")
        h_t = res.tile([E, BP], F32, tag="h_t", name="h_t")
        hcp = ctx.enter_context(tc.tile_pool(name="hcp", bufs=2))
        ohq_p = ctx.enter_context(tc.tile_pool(name="ohq_p", bufs=2))
        hlp = ctx.enter_context(tc.tile_pool(name="hlp", bufs=2))

        # ---- one-hot build ----
        def build_onehot(row, vocab):
            # step-0 onehot at base partition 0 (standalone x-side matmul)
            oh = ohp.tile([10, BP], BF16, tag="oh", name="oh")
            nc.gpsimd.dma_start(oh[:vocab, :], row_bcast_ap(tok_d, row, vocab, BP))
            nc.gpsimd.tensor_scalar(oh[:vocab, :], oh[:vocab, :], vidx[:vocab, :],
                                    None, op0=OP.is_equal)
            return oh

        def build_onehot_hc(row, vocab, hc):
            # compute-engine partition ranges must be 32-aligned, so build the
            # onehot at base partition 0 and DMA it into rows E..E+vocab
            oh = ohp.tile([10, BP], BF16, tag="oh", name="oh")
            nc.gpsimd.dma_start(oh[:vocab, :], row_bcast_ap(tok_d, row, vocab, BP))
            nc.vector.tensor_scalar(oh[:vocab, :], oh[:vocab, :],
                                    vidx[:vocab, :], None, op0=OP.is_equal)
            nc.sync.dma_start(hc[E:E + vocab, :], oh[:vocab, :])

        # ---- LSTM step (feature-land), cols = chunk range ----
        def lstm_chunks(xw_t, vocab, wc_t, hw_t, b4_t, oh, h_prev, c_prev,
                        h_new, c_new, chunks, split_to=None):
            # h_prev: None (h=0, oh = standalone onehot tile) or (hc, hl) where
            # hc = [hh; onehot] combined bf16 tile; c_prev None => c=0
            xwh, xwl = xw_t
            wch, wcl = wc_t
            hwh, hwl = hw_t
            KC = E + vocab
            ch_list = list(chunks)
            for k0 in range(0, len(ch_list), 2):
                grp = ch_list[k0:k0 + 2]
                pst = {(g, k): psp.tile([E, CH], F32, tag="ps", name="ps")
                       for g in range(4) for k in grp}
                if h_prev is None:
                    passes = [(xwh[:vocab, :], oh[:vocab, :], True),
                              (xwl[:vocab, :], oh[:vocab, :], False)]
                else:
                    hc, hl = h_prev
                    passes = [(wch[:KC, :], hc[:KC, :], True),
                              (wcl[:KC, :], hc[:KC, :], False),
                              (hwh[:, :], hl[:, :], False),
                              (hwl[:, :], hl[:, :], False)]
                np_ = len(passes)
                # weight-reuse: same stationary operand streams both chunks
                for g in range(4):
                    gs = slice(g * E, (g + 1) * E)
                    for pi, (w, rhs_t, st) in enumerate(passes):
                        wv = w[:, gs]
                        for k in grp:
                            cols = slice(k * CH, (k + 1) * CH)
                            nc.tensor.matmul(pst[(g, k)][:], wv,
                                             rhs_t[:, cols],
                                             start=st, stop=pi == np_ - 1,
                                             skip_group_check=True)
                for k in grp:
                    cols = slice(k * CH, (k + 1) * CH)
                    ig = go.tile([E, CH], F32, tag="go", name="go")
                    nc.scalar.activation(ig[:], pst[(0, k)][:], AF.Sigmoid,
                                         bias=b4_t[:, 0:1])
                    gg = go.tile([E, CH], F32, tag="go", name="go")
                    nc.scalar.activation(gg[:], pst[(2, k)][:], AF.Tanh,
                                         bias=b4_t[:, 2:3])
                    og = go.tile([E, CH], F32, tag="go", name="go")
                    nc.scalar.activation(og[:], pst[(3, k)][:], AF.Sigmoid,
                                         bias=b4_t[:, 3:4])
                    if c_prev is None:
                        nc.vector.tensor_mul(c_new[:, cols], ig[:], gg[:])
                    else:
                        fg = go.tile([E, CH], F32, tag="go", name="go")
                        nc.scalar.activation(fg[:], pst[(1, k)][:], AF.Sigmoid,
                                             bias=b4_t[:, 1:2])
                        t1 = go.tile([E, CH], F32, tag="go", name="go")
                        nc.vector.tensor_mul(t1[:], ig[:], gg[:])
                        t2 = go.tile([E, CH], F32, tag="go", name="go")
                        nc.vector.tensor_mul(t2[:], fg[:], c_prev[:, cols])
                        nc.vector.tensor_add(c_new[:, cols], t1[:], t2[:])
                    th = go.tile([E, CH], F32, tag="go", name="go")
                    nc.scalar.activation(th[:], c_new[:, cols], AF.Tanh)
                    nc.vector.tensor_mul(h_new[:, cols], og[:], th[:])
                    if split_to is not None:
                        shc, shl = split_to
                        nc.scalar.copy(shc[:E, cols], h_new[:, cols])
                        nc.vector.tensor_tensor(shl[:, cols], h_new[:, cols],
                                                shc[:E, cols], op=OP.subtract)

        # ---- encoder ----
        def encoder(key, vocab, row0, steps):
            hsp = c_prev = None
            oh = build_onehot(row0, vocab)
            for s in range(steps):
                h_new = hp.tile([E, BP], F32, tag="h", name="h")
                c_new = cp.tile([E, BP], F32, tag="c", name="c")
                split_to = None
                if s + 1 < steps:
                    hc_next = hcp.tile([112, BP], BF16, tag="hc", name="hc")
                    build_onehot_hc(row0 + s + 1, vocab, hc_next)
                    split_to = (hc_next,
                                hlp.tile([E, BP], BF16, tag="hl", name="hl"))
                lstm_chunks(xw[key], vocab, wc[key], hw[key], b4[key], oh, hsp,
                            c_prev, h_new, c_new, range(NCH), split_to)
                hsp, c_prev = split_to, c_new
                oh = None
            return h_new

        def comb_add(enc_idx, h_enc):
            for k in range(NCH):
                cols = slice(k * CH, (k + 1) * CH)
                ps = psp.tile([128, CH], F32, tag="ps", name="ps")
                nc.tensor.matmul(ps[:E, :], cwT[enc_idx][:], h_enc[:, cols],
                                 start=True, stop=True)
                if enc_idx == 0:
                    nc.vector.tensor_copy(comb_acc[:, cols], ps[:E, :])
                else:
                    nc.vector.tensor_add(comb_acc[:, cols], comb_acc[:, cols],
                                         ps[:E, :])

        c_h = encoder("ctx", 6, 0, 6)
        comb_add(0, c_h)
        m_h = encoder("m", 10, 6, 6)
        comb_add(1, m_h)
        p_h = encoder("pr", 6, 12, 3)
        comb_add(2, p_h)
        nc.scalar.activation(h_t[:], comb_acc[:], AF.Relu, bias=combb[:, 0:1])

        # ---- shared softmax/sample in head-land ----
        def softmax_sample(l2, C, nt, gum_ap, hsl, nodes_col, tok_slice,
                           ent_row, match_row):
            """l2: [128, nt, C] logits AP. Returns sampled index tile [128, nt]."""
            smax = hd.tile([128, nt], F32, tag="hs", name="hs")
            nc.vector.tensor_reduce(smax[:], l2, axis=AX.X, op=OP.max)
            sh = hd.tile([128, nt, 10], F32, tag="hd", name="hd")[:, :, :C]
            nc.vector.tensor_tensor(sh, l2, bc(smax[:], C), op=OP.subtract)
            ex = hd.tile([128, nt, 10], F32, tag="hd", name="hd")[:, :, :C]
            nc.scalar.activation(ex, sh, AF.Exp)
            S = hd.tile([128, nt], F32, tag="hs", name="hs")
            nc.vector.tensor_reduce(S[:], ex, axis=AX.X, op=OP.add)
            rec = hd.tile([128, nt], F32, tag="hs", name="hs")
            nc.vector.reciprocal(rec[:], S[:])
            probs = hd.tile([128, nt, 10], F32, tag="hd", name="hd")[:, :, :C]
            nc.vector.tensor_tensor(probs, ex, bc(rec[:], C), op=OP.mult)
            # entropy contribution sum_c (p+EPS)ln(p+EPS)
            q = hd.tile([128, nt, 10], F32, tag="hd", name="hd")[:, :, :C]
            nc.vector.tensor_single_scalar(q, probs, EPS, op=OP.add)
            lnq = hd.tile([128, nt, 10], F32, tag="hd", name="hd")[:, :, :C]
            nc.scalar.activation(lnq, q, AF.Ln)
            ql = hd.tile([128, nt, 10], F32, tag="hd", name="hd")[:, :, :C]
            nc.vector.tensor_mul(ql, q, lnq)
            ctr = hd.tile([128, nt], F32, tag="hs", name="hs")
            nc.vector.tensor_reduce(ctr[:], ql, axis=AX.X, op=OP.add)
            nc.vector.tensor_add(acc[:, ent_row, hsl], acc[:, ent_row, hsl],
                                 ctr[:])
            # categorical sample: argmax(sh + gumbel) (lnS shift cancels)
            v = hd.tile([128, nt, 10], F32, tag="hd", name="hd")[:, :, :C]
            nc.vector.tensor_tensor(v, sh, gum_ap, op=OP.add)
            vmax = hd.tile([128, nt], F32, tag="hs", name="hs")
            nc.vector.tensor_reduce(vmax[:], v, axis=AX.X, op=OP.max)
            veq = hd.tile([128, nt, 10], F32, tag="hd", name="hd")[:, :, :C]
            nc.vector.tensor_tensor(veq, v, bc(vmax[:], C), op=OP.is_equal)
            msk = hd.tile([128, nt, 10], F32, tag="hd", name="hd")[:, :, :C]
            nc.vector.tensor_mul(msk, veq, iotaC[:, :nt, :C])
            pen = hd.tile([128, nt, 10], F32, tag="hd", name="hd")[:, :, :C]
            nc.vector.tensor_scalar(pen, veq, -BIG, BIG, op0=OP.mult, op1=OP.add)
            nc.vector.tensor_add(msk, msk, pen)
            af = hd.tile([128, nt], F32, tag="hs", name="hs")
            nc.vector.tensor_reduce(af[:], msk, axis=AX.X, op=OP.min)
            nc.vector.tensor_copy(tok_slice, af[:])        # cast f32 -> i32
            # chosen prob -> nodes; greedy match via shifted-logit == 0
            aeq = hd.tile([128, nt, 10], F32, tag="hd", name="hd")[:, :, :C]
            nc.vector.tensor_tensor(aeq, iotaC[:, :nt, :C], bc(af[:], C),
                                    op=OP.is_equal)
            pa = hd.tile([128, nt, 10], F32, tag="hd", name="hd")[:, :, :C]
            nc.vector.tensor_mul(pa, probs, aeq)
            pa1 = hd.tile([128, nt], F32, tag="hs", name="hs")
            nc.vector.tensor_reduce(pa1[:], pa, axis=AX.X, op=OP.add)
            nc.scalar.activation(nodes_a[:, hsl, nodes_col], pa1[:], AF.Ln)
            ssel = hd.tile([128, nt, 10], F32, tag="hd", name="hd")[:, :, :C]
            nc.vector.tensor_mul(ssel, sh, aeq)
            ss1 = hd.tile([128, nt], F32, tag="hs", name="hs")
            nc.vector.tensor_reduce(ss1[:], ssel, axis=AX.X, op=OP.add)
            mt = hd.tile([128, nt], F32, tag="hs", name="hs")
            nc.vector.tensor_single_scalar(mt[:], ss1[:], 0.0, op=OP.is_equal)
            nc.vector.tensor_add(acc[:, match_row, hsl], acc[:, match_row, hsl],
                                 mt[:])
            return af

        # ---- term + proposal heads ----
        for half in range(2):
            hsl = slice(half * NH, (half + 1) * NH)
            ps19 = psp.tile([128, NH, 19], F32, tag="ps", name="ps")
            for t16 in range(NH):
                t = half * NH + t16
                nc.tensor.matmul(ps19[:, t16, :], h_t[:, t * 128:(t + 1) * 128],
                                 headW[:], start=True, stop=True)
            lg19 = hd19.tile([128, NH, 19], F32, tag="hd19", name="hd19")
            nc.vector.tensor_tensor(lg19[:], ps19[:], bcmid(headb[:], NH),
                                    op=OP.add)
            # term policy
            tp = hd.tile([128, NH], F32, tag="hs", name="hs")
            nc.scalar.activation(tp[:], lg19[:, :, 0], AF.Sigmoid)
            au = hd.tile([128, NH], F32, tag="hs", name="hs")
            nc.vector.tensor_tensor(au[:], ubern[:, hsl], tp[:], op=OP.is_lt)
            nc.vector.tensor_copy(aterm_a[:, hsl], au[:])   # cast -> u8
            rg = hd.tile([128, NH], F32, tag="hs", name="hs")
            nc.vector.tensor_single_scalar(rg[:], tp[:], 0.5, op=OP.is_ge)
            mt = hd.tile([128, NH], F32, tag="hs", name="hs")
            nc.vector.tensor_tensor(mt[:], rg[:], au[:], op=OP.is_equal)
            nc.vector.tensor_add(acc[:, 3, hsl], acc[:, 3, hsl], mt[:])
            # g = a*p + (1-a)*(1-p), exact fp32 mirror of the reference
            aup = hd.tile([128, NH], F32, tag="hs", name="hs")
            nc.vector.tensor_mul(aup[:], au[:], tp[:])
            nau = hd.tile([128, NH], F32, tag="hs", name="hs")
            nc.vector.tensor_scalar(nau[:], au[:], -1.0, 1.0, op0=OP.mult,
                                    op1=OP.add)
            ntp = hd.tile([128, NH], F32, tag="hs", name="hs")
            nc.vector.tensor_scalar(ntp[:], tp[:], -1.0, 1.0, op0=OP.mult,
                                    op1=OP.add)
            t2 = hd.tile([128, NH], F32, tag="hs", name="hs")
            nc.vector.tensor_mul(t2[:], nau[:], ntp[:])
            gt = hd.tile([128, NH], F32, tag="hs", name="hs")
            nc.vector.tensor_add(gt[:], aup[:], t2[:])
            nc.scalar.activation(nodes_a[:, hsl, 0], gt[:], AF.Ln)
            q = hd.tile([128, NH], F32, tag="hs", name="hs")
            nc.vector.tensor_single_scalar(q[:], tp[:], EPS, op=OP.add)
            lnq = hd.tile([128, NH], F32, tag="hs", name="hs")
            nc.scalar.activation(lnq[:], q[:], AF.Ln)
            ql = hd.tile([128, NH], F32, tag="hs", name="hs")
            nc.vector.tensor_mul(ql[:], q[:], lnq[:])
            nc.vector.tensor_add(acc[:, 0, hsl], acc[:, 0, hsl], ql[:])
            # proposal heads
            for i in range(NUM_ITEMS):
                softmax_sample(lg19[:, :, 1 + 6 * i:7 + 6 * i], 6, NH,
                               gump[:, i, hsl, :], hsl, 7 + i,
                               prop_a[:, hsl, i], 2, 5)

        # ---- utterance decode (autoregressive, pipelined in quarters) ----
        hc0 = hcp.tile([112, BP], BF16, tag="hc", name="hc")
        oh0 = ohp.tile([10, BP], BF16, tag="oh", name="oh")
        nc.vector.memset(oh0[:], 0.0)
        nc.vector.memset(oh0[0:1, :], 1.0)
        nc.sync.dma_start(hc0[E:E + 10, :], oh0[:])
        h_tl = hlp.tile([E, BP], BF16, tag="hl", name="hl")
        nc.scalar.copy(hc0[:E, :], h_t[:])
        nc.vector.tensor_tensor(h_tl[:], h_t[:], hc0[:E, :], op=OP.subtract)
        hsp = (hc0, h_tl)
        c_cur = None
        for s in range(MAX_LEN):
            rnp_s = rgp.tile([128, NT, 10], F32, tag="rnp", name="rnp")
            nc.sync.dma_start(rnp_s[:], rnp_d.ap()[:, s, :, :])
            gumu_s = rgp.tile([128, NT, 10], F32, tag="gumu", name="gumu")
            nc.sync.dma_start(gumu_s[:], gumu_d.ap()[:, s, :, :])
            h_new = hp.tile([E, BP], F32, tag="h", name="h")
            c_new = cp.tile([E, BP], F32, tag="c", name="c")
            split_to = None
            if s + 1 < MAX_LEN:
                split_to = (hcp.tile([112, BP], BF16, tag="hc", name="hc"),
                            hlp.tile([E, BP], BF16, tag="hl", name="hl"))
            for q in range(4):
                lstm_chunks(xw["up"], 10, wc["up"], hw["up"], b4["up"], None,
                            hsp, c_cur, h_new, c_new,
                            range(q * 2, q * 2 + 2), split_to)
                qsl = slice(q * NQT, (q + 1) * NQT)
                ps10 = psp.tile([128, NQT, 10], F32, tag="ps", name="ps")
                for tq in range(NQT):
                    t = q * NQT + tq
                    nc.tensor.matmul(ps10[:, tq, :],
                                     h_new[:, t * 128:(t + 1) * 128],
                                     uhW[:], start=True, stop=True)
                lg0 = hd.tile([128, NQT, 10], F32, tag="hd", name="hd")
                nc.vector.tensor_tensor(lg0[:], ps10[:], bcmid(uhb[:], NQT),
                                        op=OP.add)
                # noise blend: l2 = 0.9*l + 0.1*(lmin + (lmax-lmin)*rn)
                rmn = hd.tile([128, NQT], F32, tag="hs", name="hs")
                nc.vector.tensor_reduce(rmn[:], lg0[:], axis=AX.X, op=OP.min)
                rmx = hd.tile([128, NQT], F32, tag="hs", name="hs")
                nc.vector.tensor_reduce(rmx[:], lg0[:], axis=AX.X, op=OP.max)
                d = hd.tile([128, NQT], F32, tag="hs", name="hs")
                nc.vector.tensor_tensor(d[:], rmx[:], rmn[:], op=OP.subtract)
                n1 = hd.tile([128, NQT, 10], F32, tag="hd", name="hd")
                nc.vector.tensor_tensor(n1[:], rnp_s[:, qsl, :], bc(d[:], 10),
                                        op=OP.mult)
                noise = hd.tile([128, NQT, 10], F32, tag="hd", name="hd")
                nc.vector.tensor_tensor(noise[:], n1[:], bc(rmn[:], 10),
                                        op=OP.add)
                ns = hd.tile([128, NQT, 10], F32, tag="hd", name="hd")
                nc.vector.tensor_single_scalar(ns[:], noise[:], CORR, op=OP.mult)
                l2 = hd.tile([128, NQT, 10], F32, tag="hd", name="hd")
                nc.vector.scalar_tensor_tensor(l2[:], lg0[:], 1.0 - CORR, ns[:],
                                               op0=OP.mult, op1=OP.add)
                af = softmax_sample(l2[:], 10, NQT, gumu_s[:, qsl, :], qsl,
                                    1 + s, utt_a[:, qsl, s], 1, 4)
                if s + 1 < MAX_LEN:
                    hc_next = split_to[0]
                    rt = drp.tile([BQ], F32, tag="rt", name="rt")
                    rbase = rt[:]
                    nc.sync.dma_start(
                        bass.AP(tensor=rbase.tensor, offset=rbase.offset,
                                ap=[[1, 128], [128, NQT]]), af[:])
                    cols = slice(q * BQ, (q + 1) * BQ)
                    ohq = ohq_p.tile([10, BQ], BF16, tag="ohq", name="ohq")
                    nc.gpsimd.dma_start(
                        ohq[:],
                        bass.AP(tensor=rbase.tensor, offset=rbase.offset,
                                ap=[[0, 10], [1, BQ]]))
                    nc.vector.tensor_scalar(ohq[:], ohq[:], vidx[:], None,
                                            op0=OP.is_equal)
                    nc.sync.dma_start(hc_next[E:E + 10, cols], ohq[:])
            c_cur = c_new
            hsp = split_to

        # ---- final scalars ----
        red6 = hd.tile([128, 6], F32, tag="red6", name="red6")
        nc.vector.tensor_reduce(red6[:], acc[:], axis=AX.X, op=OP.add)
        ps_s = psp.tile([1, 6], F32, tag="ps", name="ps")
        nc.tensor.matmul(ps_s[:], ones[:], red6[:], start=True, stop=True)
        ssb = hd.tile([1, 6], F32, tag="ssb", name="ssb")
        nc.vector.tensor_copy(ssb[:], ps_s[:])
        nc.sync.dma_start(scal_o.ap(), ssb[:])

        # ---- output DMAs (head-land -> [BP, k] row-major) ----
        def out_ap(handle, k):
            return bass.AP(tensor=handle, offset=0,
                           ap=[[k, 128], [128 * k, NT], [1, k]])

        nc.sync.dma_start(out_ap(nodes_o, 10), nodes_a[:])
        nc.sync.dma_start(out_ap(utt_o, MAX_LEN), utt_a[:])
        nc.sync.dma_start(out_ap(prop_o, NUM_ITEMS), prop_a[:])
        nc.sync.dma_start(
            bass.AP(tensor=aterm_o, offset=0, ap=[[1, 128], [128, NT]]),
            aterm_a[:])

    _split_excess_sync(nc)
    return nc


# ---------------------------------------------------------------- host prep
def _rng_draws():
    if "rng" in _CACHE:
        return _CACHE["rng"]
    import jax
    import jax.numpy as jnp
    cpu = jax.devices('cpu')[0]
    with jax.default_device(cpu):
        rng = jax.random.key(1234)
        u = np.asarray(jax.random.uniform(jax.random.fold_in(rng, 0),
                                          (BTOT, 1), jnp.float32))
        raws = [np.asarray(jax.random.normal(jax.random.fold_in(rng, 100 + i),
                                             (BTOT, 10), jnp.float32))
                for i in range(MAX_LEN)]
        gu = [np.asarray(jax.random.gumbel(jax.random.fold_in(rng, 200 + i),
                                           (BTOT, 10), jnp.float32))
              for i in range(MAX_LEN)]
        gp = [np.asarray(jax.random.gumbel(jax.random.fold_in(rng, 300 + i),
                                           (BTOT, 6), jnp.float32))
              for i in range(NUM_ITEMS)]
    rn = []
    for r in raws:
        nmin = r.min(axis=1, keepdims=True)
        nmax = r.max(axis=1, keepdims=True)
        rn.append(((r - nmin) / (nmax - nmin)).astype(np.float32))
    _CACHE["rng"] = (u, rn, gu, gp)
    return _CACHE["rng"]


def _hl(x):
    """[BP, k] (or [BP]) -> head-land [128, NT, k] / [128, NT]."""
    if x.ndim == 1:
        return np.ascontiguousarray(x.reshape(NT, 128).T)
    return np.ascontiguousarray(
        x.reshape(NT, 128, x.shape[1]).transpose(1, 0, 2))


def _bf16_split(x):
    import ml_dtypes
    bf16 = ml_dtypes.bfloat16
    xh = x.astype(bf16)
    xl = (x - xh.astype(np.float32)).astype(bf16)
    return np.ascontiguousarray(xh), np.ascontiguousarray(xl)


def _prep_in_maps(pool, utility, m_prev, prev_proposal, params):
    f32 = np.float32
    P = {k: np.asarray(v, f32) for k, v in params.items()}
    toks = np.stack([np.asarray(t, np.int64)[:, j].astype(f32)
                     for t, jr in [(pool, range(3)), (utility, range(3)),
                                   (m_prev, range(6)), (prev_proposal, range(3))]
                     for j in jr])  # [15, BTOT]

    wmap = {
        "b4_ctx": np.ascontiguousarray((P["ctx_bih"] + P["ctx_bhh"]).reshape(4, E).T),
        "b4_m": np.ascontiguousarray((P["utt_bih"] + P["utt_bhh"]).reshape(4, E).T),
        "b4_pr": np.ascontiguousarray((P["prop_bih"] + P["prop_bhh"]).reshape(4, E).T),
        "b4_up": np.ascontiguousarray((P["up_bih"] + P["up_bhh"]).reshape(4, E).T),
        "cwT": np.ascontiguousarray(P["comb_W"].T),
        "combb": P["comb_b"].reshape(E, 1),
        "headW": np.concatenate([P["term_W"].T] +
                                [P["pp_W"][i].T for i in range(NUM_ITEMS)], axis=1),
        "headb": np.broadcast_to(
            np.concatenate([P["term_b"]] + [P["pp_b"][i] for i in range(NUM_ITEMS)]),
            (128, 19)).copy(),
        "uhW": np.ascontiguousarray(P["up_h1_W"].T),
        "uhb": np.broadcast_to(P["up_h1_b"], (128, 10)).copy(),
    }
    wmap = {k: np.ascontiguousarray(v, dtype=f32) for k, v in wmap.items()}
    for key, xwm, hwm in [
            ("ctx", P["emb_ctx"] @ P["ctx_Wih"].T, P["ctx_Whh"].T),
            ("m", P["emb_utt"] @ P["utt_Wih"].T, P["utt_Whh"].T),
            ("pr", P["emb_ctx"] @ P["prop_Wih"].T, P["prop_Whh"].T),
            ("up", P["up_emb"] @ P["up_Wih"].T, P["up_Whh"].T)]:
        xh, xl = _bf16_split(np.ascontiguousarray(xwm, dtype=f32))
        hh, hl = _bf16_split(np.ascontiguousarray(hwm, dtype=f32))
        wmap[f"xwh_{key}"], wmap[f"xwl_{key}"] = xh, xl
        wmap[f"hwh_{key}"], wmap[f"hwl_{key}"] = hh, hl
        wmap[f"wch_{key}"] = np.concatenate([hh, xh], axis=0)
        wmap[f"wcl_{key}"] = np.concatenate([hl, xl], axis=0)

    u, rn, gu, gp = _rng_draws()
    in_maps = []
    for c in range(NCORES):
        sl = slice(c * BP, (c + 1) * BP)
        m = dict(wmap)
        m["tok"] = np.ascontiguousarray(toks[:, sl])
        m["ubern"] = _hl(u[sl, 0])
        m["rnp"] = np.ascontiguousarray(
            np.stack([_hl(rn[i][sl]) for i in range(MAX_LEN)], axis=1))
        m["gumu"] = np.ascontiguousarray(
            np.stack([_hl(gu[i][sl]) for i in range(MAX_LEN)], axis=1))
        m["gump"] = np.ascontiguousarray(
            np.stack([_hl(gp[i][sl]) for i in range(NUM_ITEMS)], axis=1))
        in_maps.append(m)
    return in_maps


def _get_nc():
    if "nc" not in _CACHE:
        _CACHE["nc"] = _build_program()
    return _CACHE["nc"]


def _run(in_maps, trace=False, trace_kwargs=None):
    from concourse.bass_utils import run_bass_kernel_spmd
    nc = _get_nc()
    kw = {}
    if trace:
        kw["trace"] = True
        if trace_kwargs:
            kw["trace_kwargs"] = trace_kwargs
    return run_bass_kernel_spmd(nc, in_maps, core_ids=list(range(NCORES)), **kw)


def _assemble(results):
    nodes = np.concatenate([r["nodes_o"] for r in results], axis=0)
    a_term = np.concatenate([r["aterm_o"] for r in results])[:, None]
    utterance = np.concatenate([r["utt_o"] for r in results], axis=0)
    proposal = np.concatenate([r["prop_o"] for r in results], axis=0)
    scal = np.stack([r["scal_o"][0] for r in results]).astype(np.float64)
    s_term, s_utt, s_prop, m_term, m_utt, m_prop = scal.sum(axis=0)
    entropy_loss = np.float32(0.05 * s_term + 0.001 * s_utt + 0.05 * s_prop)
    return (nodes.astype(np.float32), a_term.astype(np.uint8),
            utterance.astype(np.int32), proposal.astype(np.int32),
            entropy_loss, np.int32(round(m_term)), np.int32(round(m_utt)),
            MAX_LEN * BTOT, np.int32(round(m_prop)), NUM_ITEMS * BTOT)


def kernel(pool, utility, m_prev, prev_proposal, params):
    in_maps = _prep_in_maps(pool, utility, m_prev, prev_proposal, params)
    res = _run(in_maps, trace=False)
    return _assemble(res.results)


# revision 17
# speedup vs baseline: 1.6622x; 1.1291x over previous
"""Trainium2 Bass kernel for nn_AgentModel (negotiation agent forward pass).

Self-contained: takes FULL inputs, shards batch across 8 NeuronCores (pure
data parallel), runs one fp32 Bass/Tile program per core, gathers outputs.

Numerical strategy: all jax.random draws in the reference are input-
independent (uniform/normal/gumbel with fixed keys+shapes), so they are
precomputed on host with jax-CPU and fed to the device. All sampling
decisions (categorical argmax, greedy argmax, bernoulli compare) are made
from exact fp32 arithmetic on shifted logits -- the coarse ACT exp/ln LUTs
only touch output values (nodes/entropy) whose tolerances are loose.
"""
import numpy as np

E = 100
EPS = 1e-8
CORR = 0.1
MAX_LEN = 6
NUM_ITEMS = 3
NCORES = 8
BTOT = 32768
BP = BTOT // NCORES          # 4096 per core
NT = BP // 128               # 32 batch tiles of 128
NH = NT // 2                 # 16 tiles per half
BH = BP // 2                 # 2048 per half
NQT = NT // 4                # 8 tiles per quarter
BQ = BP // 4                 # 1024 per quarter
CH = 512                     # matmul N chunk (fp32 limit / PSUM bank)
NCH = BP // CH               # 8 chunks
BIG = 1e9

_CACHE = {}


# ---------------------------------------------------------------- sync fixup
def _split_excess_sync(nc, wait_limit=1, update_limit=1):
    """walrus rejects >1 sync wait/update per instruction in this toolchain;
    split extras onto adjacent same-engine NoOps (engines execute in order)."""
    import concourse.mybir as mybir
    n = 0
    for f in nc.m.functions:
        for bb in f.blocks:
            new_insts = []
            for ins in bb.instructions:
                si = ins.sync_info
                waits = list(si.on_wait) if si and si.on_wait else []
                updates = list(si.on_update) if si and si.on_update else []
                pre, post = [], []
                while len(waits) > wait_limit:
                    chunk, waits = waits[:wait_limit], waits[wait_limit:]
                    n += 1
                    pre.append(mybir.InstNoOp(
                        name=f"{ins.name}-ws{len(pre)}", engine=ins.engine,
                        ins=[], outs=[],
                        sync_info=mybir.SyncInfo(on_wait=chunk, on_update=[])))
                while len(updates) > update_limit:
                    chunk, updates = updates[:update_limit], updates[update_limit:]
                    n += 1
                    post.append(mybir.InstNoOp(
                        name=f"{ins.name}-us{len(post)}", engine=ins.engine,
                        ins=[], outs=[],
                        sync_info=mybir.SyncInfo(on_wait=[], on_update=chunk)))
                if si is not None:
                    si.on_wait = waits
                    si.on_update = updates
                new_insts.extend(pre)
                new_insts.append(ins)
                new_insts.extend(post)
            if len(new_insts) != len(bb.instructions):
                bb.instructions[:] = new_insts
    return n


# ---------------------------------------------------------------- program
def _build_program():
    import concourse.bass as bass
    import concourse.tile as tile
    from concourse import mybir
    from contextlib import ExitStack

    F32 = mybir.dt.float32
    I32 = mybir.dt.int32
    U8 = mybir.dt.uint8
    AF = mybir.ActivationFunctionType
    OP = mybir.AluOpType
    AX = mybir.AxisListType

    nc = bass.Bass("TRN2", target_bir_lowering=False, debug=False,
                   num_devices=NCORES)

    def dram_in(name, shape, dt=F32):
        return nc.dram_tensor(name, shape, dt, kind="ExternalInput")

    def dram_out(name, shape, dt=F32):
        return nc.dram_tensor(name, shape, dt, kind="ExternalOutput")

    tok_d = dram_in("tok", [15, BP])
    ubern_d = dram_in("ubern", [128, NT])
    rnp_d = dram_in("rnp", [128, MAX_LEN, NT, 10])
    gumu_d = dram_in("gumu", [128, MAX_LEN, NT, 10])
    gump_d = dram_in("gump", [128, NUM_ITEMS, NT, 6])
    BF16 = mybir.dt.bfloat16
    xw_d = {k: [dram_in(f"xw{p}_{k}", [v, 4 * E], BF16) for p in "hl"]
            for k, v in [("ctx", 6), ("m", 10), ("pr", 6), ("up", 10)]}
    hw_d = {k: [dram_in(f"hw{p}_{k}", [E, 4 * E], BF16) for p in "hl"]
            for k in ("ctx", "m", "pr", "up")}
    VOC = {"ctx": 6, "m": 10, "pr": 6, "up": 10}
    wc_d = {k: [dram_in(f"wc{p}_{k}", [E + VOC[k], 4 * E], BF16) for p in "hl"]
            for k in ("ctx", "m", "pr", "up")}
    b4_d = {k: dram_in(f"b4_{k}", [E, 4]) for k in ("ctx", "m", "pr", "up")}
    cwT_d = dram_in("cwT", [300, E])
    combb_d = dram_in("combb", [E, 1])
    headW_d = dram_in("headW", [E, 19])
    headb_d = dram_in("headb", [128, 19])
    uhW_d = dram_in("uhW", [E, 10])
    uhb_d = dram_in("uhb", [128, 10])

    nodes_o = dram_out("nodes_o", [BP, 10])
    aterm_o = dram_out("aterm_o", [BP], U8)
    utt_o = dram_out("utt_o", [BP, MAX_LEN], I32)
    prop_o = dram_out("prop_o", [BP, NUM_ITEMS], I32)
    scal_o = dram_out("scal_o", [1, 6])

    def row_bcast_ap(handle, row, nrows, ncols):
        """DRAM [R, C] row -> AP broadcasting that row across nrows partitions."""
        return bass.AP(tensor=handle, offset=row * ncols,
                       ap=[[0, nrows], [1, ncols]])

    def bc(ap3, n):
        """[128, X] AP -> [128, X, n] via trailing 0-stride broadcast."""
        return ap3.to_broadcast(list(ap3.shape) + [n])

    def bcmid(ap2, n):
        """[128, X] AP -> [128, n, X] via middle 0-stride broadcast."""
        return bass.AP(tensor=ap2.tensor, offset=ap2.offset,
                       ap=[list(ap2.ap[0]), [0, n], list(ap2.ap[1])])

    ctx = ExitStack()
    with tile.TileContext(nc) as tc, ctx:
        wsb = ctx.enter_context(tc.tile_pool(name="wsb", bufs=1))
        res = ctx.enter_context(tc.tile_pool(name="res", bufs=1))
        ohp = ctx.enter_context(tc.tile_pool(name="ohp", bufs=1))
        hp = ctx.enter_context(tc.tile_pool(name="hp", bufs=2))
        cp = ctx.enter_context(tc.tile_pool(name="cp", bufs=2))
        go = ctx.enter_context(tc.tile_pool(name="go", bufs=6))
        hd = ctx.enter_context(tc.tile_pool(name="hd", bufs=18))
        rgp = ctx.enter_context(tc.tile_pool(name="rgp", bufs=2))
        hd19 = ctx.enter_context(tc.tile_pool(name="hd19", bufs=2))
        psp = ctx.enter_context(tc.tile_pool(name="psp", bufs=8, space="PSUM"))
        drp = ctx.enter_context(tc.tile_pool(name="drp", bufs=4, space="DRAM"))

        # ---- load weights ----
        def wload(dram, shape, tag):
            t = wsb.tile(shape, F32, tag=tag, name=tag)
            nc.sync.dma_start(t[:], dram.ap())
            return t

        def wload2(pair, shape, tag):
            out = []
            for p, dram in zip("hl", pair):
                t = wsb.tile(shape, BF16, tag=f"{tag}{p}", name=f"{tag}{p}")
                nc.sync.dma_start(t[:], dram.ap())
                out.append(t)
            return out

        xw = {k: wload2(d, [d[0].shape[0], 4 * E], f"xw{k}") for k, d in xw_d.items()}
        hw = {k: wload2(d, [E, 4 * E], f"hw{k}") for k, d in hw_d.items()}
        wc = {k: wload2(d, [E + VOC[k], 4 * E], f"wc{k}") for k, d in wc_d.items()}
        b4 = {k: wload(d, [E, 4], f"b4{k}") for k, d in b4_d.items()}
        cwT = [wsb.tile([E, E], F32, tag=f"cwT{i}", name=f"cwT{i}") for i in range(3)]
        for i in range(3):
            nc.sync.dma_start(cwT[i][:], cwT_d.ap()[i * E:(i + 1) * E, :])
        combb = wload(combb_d, [E, 1], "combb")
        headW = wload(headW_d, [E, 19], "headW")
        headb = wload(headb_d, [128, 19], "headb")
        uhW = wload(uhW_d, [E, 10], "uhW")
        uhb = wload(uhb_d, [128, 10], "uhb")

        # ---- resident data ----
        ubern = res.tile([128, NT], F32, tag="ubern", name="ubern")
        nc.sync.dma_start(ubern[:], ubern_d.ap())
        gump = res.tile([128, NUM_ITEMS, NT, 6], F32, tag="gump", name="gump")
        nc.sync.dma_start(gump[:], gump_d.ap())

        # iota over class dim [128, NH, 10] and per-partition vocab index [10,1]
        iotaI = res.tile([128, NH, 10], I32, tag="iotaI", name="iotaI")
        nc.gpsimd.iota(iotaI[:], pattern=[[0, NH], [1, 10]], base=0,
                       channel_multiplier=0)
        iotaC = res.tile([128, NH, 10], F32, tag="iotaC", name="iotaC")
        nc.vector.tensor_copy(iotaC[:], iotaI[:])
        vidxI = res.tile([10, 1], I32, tag="vidxI", name="vidxI")
        nc.gpsimd.iota(vidxI[:], pattern=[[0, 1]], base=0, channel_multiplier=1)
        vidx = res.tile([10, 1], F32, tag="vidx", name="vidx")
        nc.vector.tensor_copy(vidx[:], vidxI[:])

        ones = res.tile([128, 1], F32, tag="ones", name="ones")
        nc.vector.memset(ones[:], 1.0)

        # accumulators: rows 0..5 = S_term, S_utt, S_prop, m_term, m_utt, m_prop
        acc = res.tile([128, 6, NT], F32, tag="acc", name="acc")
        nc.vector.memset(acc[:], 0.0)
        nodes_a = res.tile([128, NT, 10], F32, tag="nodes_a", name="nodes_a")
        utt_a = res.tile([128, NT, MAX_LEN], I32, tag="utt_a", name="utt_a")
        prop_a = res.tile([128, NT, NUM_ITEMS], I32, tag="prop_a", name="prop_a")
        aterm_a = res.tile([128, NT], U8, tag="aterm_a", name="aterm_a")

        comb_acc = res.tile([E, BP], F32, tag="comb_acc", name="comb_acc")
        h_t = res.tile([E, BP], F32, tag="h_t", name="h_t")
        hcp = ctx.enter_context(tc.tile_pool(name="hcp", bufs=2))
        ohq_p = ctx.enter_context(tc.tile_pool(name="ohq_p", bufs=2))
        hlp = ctx.enter_context(tc.tile_pool(name="hlp", bufs=2))

        # ---- one-hot build ----
        def build_onehot(row, vocab):
            # step-0 onehot at base partition 0 (standalone x-side matmul)
            oh = ohp.tile([10, BP], BF16, tag="oh", name="oh")
            nc.gpsimd.dma_start(oh[:vocab, :], row_bcast_ap(tok_d, row, vocab, BP))
            nc.vector.tensor_scalar(oh[:vocab, :], oh[:vocab, :], vidx[:vocab, :],
                                    None, op0=OP.is_equal)
            return oh

        def build_onehot_hc(row, vocab, hc):
            # compute-engine partition ranges must be 32-aligned, so build the
            # onehot at base partition 0 and DMA it into rows E..E+vocab
            oh = ohp.tile([10, BP], BF16, tag="oh", name="oh")
            nc.gpsimd.dma_start(oh[:vocab, :], row_bcast_ap(tok_d, row, vocab, BP))
            nc.vector.tensor_scalar(oh[:vocab, :], oh[:vocab, :],
                                    vidx[:vocab, :], None, op0=OP.is_equal)
            nc.sync.dma_start(hc[E:E + vocab, :], oh[:vocab, :])

        # ---- LSTM step (feature-land), cols = chunk range ----
        def lstm_chunks(xw_t, vocab, wc_t, hw_t, b4_t, oh, h_prev, c_prev,
                        h_new, c_new, chunks, split_to=None):
            # h_prev: None (h=0, oh = standalone onehot tile) or (hc, hl) where
            # hc = [hh; onehot] combined bf16 tile; c_prev None => c=0
            xwh, xwl = xw_t
            wch, wcl = wc_t
            hwh, hwl = hw_t
            KC = E + vocab
            ch_list = list(chunks)
            for k0 in range(0, len(ch_list), 2):
                grp = ch_list[k0:k0 + 2]
                pst = {(g, k): psp.tile([E, CH], F32, tag="ps", name="ps")
                       for g in range(4) for k in grp}
                if h_prev is None:
                    passes = [(xwh[:vocab, :], oh[:vocab, :], True),
                              (xwl[:vocab, :], oh[:vocab, :], False)]
                else:
                    hc, hl = h_prev
                    passes = [(wch[:KC, :], hc[:KC, :], True),
                              (wcl[:KC, :], hc[:KC, :], False),
                              (hwh[:, :], hl[:, :], False),
                              (hwl[:, :], hl[:, :], False)]
                np_ = len(passes)
                # weight-reuse: same stationary operand streams both chunks
                for g in range(4):
                    gs = slice(g * E, (g + 1) * E)
                    for pi, (w, rhs_t, st) in enumerate(passes):
                        wv = w[:, gs]
                        for k in grp:
                            cols = slice(k * CH, (k + 1) * CH)
                            nc.tensor.matmul(pst[(g, k)][:], wv,
                                             rhs_t[:, cols],
                                             start=st, stop=pi == np_ - 1,
                                             skip_group_check=True)
                for k in grp:
                    cols = slice(k * CH, (k + 1) * CH)
                    ig = go.tile([E, CH], F32, tag="go", name="go")
                    nc.scalar.activation(ig[:], pst[(0, k)][:], AF.Sigmoid,
                                         bias=b4_t[:, 0:1])
                    gg = go.tile([E, CH], F32, tag="go", name="go")
                    nc.scalar.activation(gg[:], pst[(2, k)][:], AF.Tanh,
                                         bias=b4_t[:, 2:3])
                    og = go.tile([E, CH], F32, tag="go", name="go")
                    nc.scalar.activation(og[:], pst[(3, k)][:], AF.Sigmoid,
                                         bias=b4_t[:, 3:4])
                    if c_prev is None:
                        nc.vector.tensor_mul(c_new[:, cols], ig[:], gg[:])
                    else:
                        fg = go.tile([E, CH], F32, tag="go", name="go")
                        nc.scalar.activation(fg[:], pst[(1, k)][:], AF.Sigmoid,
                                             bias=b4_t[:, 1:2])
                        t1 = go.tile([E, CH], F32, tag="go", name="go")
                        nc.vector.tensor_mul(t1[:], ig[:], gg[:])
                        t2 = go.tile([E, CH], F32, tag="go", name="go")
                        nc.vector.tensor_mul(t2[:], fg[:], c_prev[:, cols])
                        nc.vector.tensor_add(c_new[:, cols], t1[:], t2[:])
                    th = go.tile([E, CH], F32, tag="go", name="go")
                    nc.scalar.activation(th[:], c_new[:, cols], AF.Tanh)
                    nc.vector.tensor_mul(h_new[:, cols], og[:], th[:])
                    if split_to is not None:
                        shc, shl = split_to
                        nc.scalar.copy(shc[:E, cols], h_new[:, cols])
                        nc.vector.tensor_tensor(shl[:, cols], h_new[:, cols],
                                                shc[:E, cols], op=OP.subtract)

        # ---- encoder ----
        def encoder(key, vocab, row0, steps):
            hsp = c_prev = None
            oh = build_onehot(row0, vocab)
            for s in range(steps):
                h_new = hp.tile([E, BP], F32, tag="h", name="h")
                c_new = cp.tile([E, BP], F32, tag="c", name="c")
                split_to = None
                if s + 1 < steps:
                    hc_next = hcp.tile([112, BP], BF16, tag="hc", name="hc")
                    build_onehot_hc(row0 + s + 1, vocab, hc_next)
                    split_to = (hc_next,
                                hlp.tile([E, BP], BF16, tag="hl", name="hl"))
                lstm_chunks(xw[key], vocab, wc[key], hw[key], b4[key], oh, hsp,
                            c_prev, h_new, c_new, range(NCH), split_to)
                hsp, c_prev = split_to, c_new
                oh = None
            return h_new

        def comb_add(enc_idx, h_enc):
            for k in range(NCH):
                cols = slice(k * CH, (k + 1) * CH)
                ps = psp.tile([128, CH], F32, tag="ps", name="ps")
                nc.tensor.matmul(ps[:E, :], cwT[enc_idx][:], h_enc[:, cols],
                                 start=True, stop=True)
                if enc_idx == 0:
                    nc.vector.tensor_copy(comb_acc[:, cols], ps[:E, :])
                else:
                    nc.vector.tensor_add(comb_acc[:, cols], comb_acc[:, cols],
                                         ps[:E, :])

        c_h = encoder("ctx", 6, 0, 6)
        comb_add(0, c_h)
        m_h = encoder("m", 10, 6, 6)
        comb_add(1, m_h)
        p_h = encoder("pr", 6, 12, 3)
        comb_add(2, p_h)
        nc.scalar.activation(h_t[:], comb_acc[:], AF.Relu, bias=combb[:, 0:1])

        # ---- shared softmax/sample in head-land ----
        def softmax_sample(l2, C, nt, gum_ap, hsl, nodes_col, tok_slice,
                           ent_row, match_row):
            """l2: [128, nt, C] logits AP. Returns sampled index tile [128, nt]."""
            smax = hd.tile([128, nt], F32, tag="hs", name="hs")
            nc.vector.tensor_reduce(smax[:], l2, axis=AX.X, op=OP.max)
            sh = hd.tile([128, nt, 10], F32, tag="hd", name="hd")[:, :, :C]
            nc.vector.tensor_tensor(sh, l2, bc(smax[:], C), op=OP.subtract)
            ex = hd.tile([128, nt, 10], F32, tag="hd", name="hd")[:, :, :C]
            nc.scalar.activation(ex, sh, AF.Exp)
            S = hd.tile([128, nt], F32, tag="hs", name="hs")
            nc.vector.tensor_reduce(S[:], ex, axis=AX.X, op=OP.add)
            rec = hd.tile([128, nt], F32, tag="hs", name="hs")
            nc.vector.reciprocal(rec[:], S[:])
            probs = hd.tile([128, nt, 10], F32, tag="hd", name="hd")[:, :, :C]
            nc.vector.tensor_tensor(probs, ex, bc(rec[:], C), op=OP.mult)
            # entropy contribution sum_c (p+EPS)ln(p+EPS)
            q = hd.tile([128, nt, 10], F32, tag="hd", name="hd")[:, :, :C]
            nc.vector.tensor_single_scalar(q, probs, EPS, op=OP.add)
            lnq = hd.tile([128, nt, 10], F32, tag="hd", name="hd")[:, :, :C]
            nc.scalar.activation(lnq, q, AF.Ln)
            ql = hd.tile([128, nt, 10], F32, tag="hd", name="hd")[:, :, :C]
            nc.vector.tensor_mul(ql, q, lnq)
            ctr = hd.tile([128, nt], F32, tag="hs", name="hs")
            nc.vector.tensor_reduce(ctr[:], ql, axis=AX.X, op=OP.add)
            nc.vector.tensor_add(acc[:, ent_row, hsl], acc[:, ent_row, hsl],
                                 ctr[:])
            # categorical sample: argmax(sh + gumbel) (lnS shift cancels)
            v = hd.tile([128, nt, 10], F32, tag="hd", name="hd")[:, :, :C]
            nc.vector.tensor_tensor(v, sh, gum_ap, op=OP.add)
            vmax = hd.tile([128, nt], F32, tag="hs", name="hs")
            nc.vector.tensor_reduce(vmax[:], v, axis=AX.X, op=OP.max)
            veq = hd.tile([128, nt, 10], F32, tag="hd", name="hd")[:, :, :C]
            nc.vector.tensor_tensor(veq, v, bc(vmax[:], C), op=OP.is_equal)
            msk = hd.tile([128, nt, 10], F32, tag="hd", name="hd")[:, :, :C]
            nc.vector.tensor_mul(msk, veq, iotaC[:, :nt, :C])
            pen = hd.tile([128, nt, 10], F32, tag="hd", name="hd")[:, :, :C]
            nc.vector.tensor_scalar(pen, veq, -BIG, BIG, op0=OP.mult, op1=OP.add)
            nc.vector.tensor_add(msk, msk, pen)
            af = hd.tile([128, nt], F32, tag="hs", name="hs")
            nc.vector.tensor_reduce(af[:], msk, axis=AX.X, op=OP.min)
            nc.vector.tensor_copy(tok_slice, af[:])        # cast f32 -> i32
            # chosen prob -> nodes; greedy match via shifted-logit == 0
            aeq = hd.tile([128, nt, 10], F32, tag="hd", name="hd")[:, :, :C]
            nc.vector.tensor_tensor(aeq, iotaC[:, :nt, :C], bc(af[:], C),
                                    op=OP.is_equal)
            pa = hd.tile([128, nt, 10], F32, tag="hd", name="hd")[:, :, :C]
            nc.vector.tensor_mul(pa, probs, aeq)
            pa1 = hd.tile([128, nt], F32, tag="hs", name="hs")
            nc.vector.tensor_reduce(pa1[:], pa, axis=AX.X, op=OP.add)
            nc.scalar.activation(nodes_a[:, hsl, nodes_col], pa1[:], AF.Ln)
            ssel = hd.tile([128, nt, 10], F32, tag="hd", name="hd")[:, :, :C]
            nc.vector.tensor_mul(ssel, sh, aeq)
            ss1 = hd.tile([128, nt], F32, tag="hs", name="hs")
            nc.vector.tensor_reduce(ss1[:], ssel, axis=AX.X, op=OP.add)
            mt = hd.tile([128, nt], F32, tag="hs", name="hs")
            nc.vector.tensor_single_scalar(mt[:], ss1[:], 0.0, op=OP.is_equal)
            nc.vector.tensor_add(acc[:, match_row, hsl], acc[:, match_row, hsl],
                                 mt[:])
            return af

        # ---- term + proposal heads ----
        for half in range(2):
            hsl = slice(half * NH, (half + 1) * NH)
            ps19 = psp.tile([128, NH, 19], F32, tag="ps", name="ps")
            for t16 in range(NH):
                t = half * NH + t16
                nc.tensor.matmul(ps19[:, t16, :], h_t[:, t * 128:(t + 1) * 128],
                                 headW[:], start=True, stop=True)
            lg19 = hd19.tile([128, NH, 19], F32, tag="hd19", name="hd19")
            nc.vector.tensor_tensor(lg19[:], ps19[:], bcmid(headb[:], NH),
                                    op=OP.add)
            # term policy
            tp = hd.tile([128, NH], F32, tag="hs", name="hs")
            nc.scalar.activation(tp[:], lg19[:, :, 0], AF.Sigmoid)
            au = hd.tile([128, NH], F32, tag="hs", name="hs")
            nc.vector.tensor_tensor(au[:], ubern[:, hsl], tp[:], op=OP.is_lt)
            nc.vector.tensor_copy(aterm_a[:, hsl], au[:])   # cast -> u8
            rg = hd.tile([128, NH], F32, tag="hs", name="hs")
            nc.vector.tensor_single_scalar(rg[:], tp[:], 0.5, op=OP.is_ge)
            mt = hd.tile([128, NH], F32, tag="hs", name="hs")
            nc.vector.tensor_tensor(mt[:], rg[:], au[:], op=OP.is_equal)
            nc.vector.tensor_add(acc[:, 3, hsl], acc[:, 3, hsl], mt[:])
            # g = a*p + (1-a)*(1-p), exact fp32 mirror of the reference
            aup = hd.tile([128, NH], F32, tag="hs", name="hs")
            nc.vector.tensor_mul(aup[:], au[:], tp[:])
            nau = hd.tile([128, NH], F32, tag="hs", name="hs")
            nc.vector.tensor_scalar(nau[:], au[:], -1.0, 1.0, op0=OP.mult,
                                    op1=OP.add)
            ntp = hd.tile([128, NH], F32, tag="hs", name="hs")
            nc.vector.tensor_scalar(ntp[:], tp[:], -1.0, 1.0, op0=OP.mult,
                                    op1=OP.add)
            t2 = hd.tile([128, NH], F32, tag="hs", name="hs")
            nc.vector.tensor_mul(t2[:], nau[:], ntp[:])
            gt = hd.tile([128, NH], F32, tag="hs", name="hs")
            nc.vector.tensor_add(gt[:], aup[:], t2[:])
            nc.scalar.activation(nodes_a[:, hsl, 0], gt[:], AF.Ln)
            q = hd.tile([128, NH], F32, tag="hs", name="hs")
            nc.vector.tensor_single_scalar(q[:], tp[:], EPS, op=OP.add)
            lnq = hd.tile([128, NH], F32, tag="hs", name="hs")
            nc.scalar.activation(lnq[:], q[:], AF.Ln)
            ql = hd.tile([128, NH], F32, tag="hs", name="hs")
            nc.vector.tensor_mul(ql[:], q[:], lnq[:])
            nc.vector.tensor_add(acc[:, 0, hsl], acc[:, 0, hsl], ql[:])
            # proposal heads
            for i in range(NUM_ITEMS):
                softmax_sample(lg19[:, :, 1 + 6 * i:7 + 6 * i], 6, NH,
                               gump[:, i, hsl, :], hsl, 7 + i,
                               prop_a[:, hsl, i], 2, 5)

        # ---- utterance decode (autoregressive, pipelined in quarters) ----
        hc0 = hcp.tile([112, BP], BF16, tag="hc", name="hc")
        oh0 = ohp.tile([10, BP], BF16, tag="oh", name="oh")
        nc.vector.memset(oh0[:], 0.0)
        nc.vector.memset(oh0[0:1, :], 1.0)
        nc.sync.dma_start(hc0[E:E + 10, :], oh0[:])
        h_tl = hlp.tile([E, BP], BF16, tag="hl", name="hl")
        nc.scalar.copy(hc0[:E, :], h_t[:])
        nc.vector.tensor_tensor(h_tl[:], h_t[:], hc0[:E, :], op=OP.subtract)
        hsp = (hc0, h_tl)
        c_cur = None
        for s in range(MAX_LEN):
            rnp_s = rgp.tile([128, NT, 10], F32, tag="rnp", name="rnp")
            nc.sync.dma_start(rnp_s[:], rnp_d.ap()[:, s, :, :])
            gumu_s = rgp.tile([128, NT, 10], F32, tag="gumu", name="gumu")
            nc.sync.dma_start(gumu_s[:], gumu_d.ap()[:, s, :, :])
            h_new = hp.tile([E, BP], F32, tag="h", name="h")
            c_new = cp.tile([E, BP], F32, tag="c", name="c")
            split_to = None
            if s + 1 < MAX_LEN:
                split_to = (hcp.tile([112, BP], BF16, tag="hc", name="hc"),
                            hlp.tile([E, BP], BF16, tag="hl", name="hl"))
            for q in range(4):
                lstm_chunks(xw["up"], 10, wc["up"], hw["up"], b4["up"], None,
                            hsp, c_cur, h_new, c_new,
                            range(q * 2, q * 2 + 2), split_to)
                qsl = slice(q * NQT, (q + 1) * NQT)
                ps10 = psp.tile([128, NQT, 10], F32, tag="ps", name="ps")
                for tq in range(NQT):
                    t = q * NQT + tq
                    nc.tensor.matmul(ps10[:, tq, :],
                                     h_new[:, t * 128:(t + 1) * 128],
                                     uhW[:], start=True, stop=True)
                lg0 = hd.tile([128, NQT, 10], F32, tag="hd", name="hd")
                nc.vector.tensor_tensor(lg0[:], ps10[:], bcmid(uhb[:], NQT),
                                        op=OP.add)
                # noise blend: l2 = 0.9*l + 0.1*(lmin + (lmax-lmin)*rn)
                rmn = hd.tile([128, NQT], F32, tag="hs", name="hs")
                nc.vector.tensor_reduce(rmn[:], lg0[:], axis=AX.X, op=OP.min)
                rmx = hd.tile([128, NQT], F32, tag="hs", name="hs")
                nc.vector.tensor_reduce(rmx[:], lg0[:], axis=AX.X, op=OP.max)
                d = hd.tile([128, NQT], F32, tag="hs", name="hs")
                nc.vector.tensor_tensor(d[:], rmx[:], rmn[:], op=OP.subtract)
                n1 = hd.tile([128, NQT, 10], F32, tag="hd", name="hd")
                nc.vector.tensor_tensor(n1[:], rnp_s[:, qsl, :], bc(d[:], 10),
                                        op=OP.mult)
                noise = hd.tile([128, NQT, 10], F32, tag="hd", name="hd")
                nc.vector.tensor_tensor(noise[:], n1[:], bc(rmn[:], 10),
                                        op=OP.add)
                ns = hd.tile([128, NQT, 10], F32, tag="hd", name="hd")
                nc.vector.tensor_single_scalar(ns[:], noise[:], CORR, op=OP.mult)
                l2 = hd.tile([128, NQT, 10], F32, tag="hd", name="hd")
                nc.vector.scalar_tensor_tensor(l2[:], lg0[:], 1.0 - CORR, ns[:],
                                               op0=OP.mult, op1=OP.add)
                af = softmax_sample(l2[:], 10, NQT, gumu_s[:, qsl, :], qsl,
                                    1 + s, utt_a[:, qsl, s], 1, 4)
                if s + 1 < MAX_LEN:
                    hc_next = split_to[0]
                    rt = drp.tile([BQ], F32, tag="rt", name="rt")
                    rbase = rt[:]
                    nc.sync.dma_start(
                        bass.AP(tensor=rbase.tensor, offset=rbase.offset,
                                ap=[[1, 128], [128, NQT]]), af[:])
                    cols = slice(q * BQ, (q + 1) * BQ)
                    ohq = ohq_p.tile([10, BQ], BF16, tag="ohq", name="ohq")
                    nc.gpsimd.dma_start(
                        ohq[:],
                        bass.AP(tensor=rbase.tensor, offset=rbase.offset,
                                ap=[[0, 10], [1, BQ]]))
                    nc.vector.tensor_scalar(ohq[:], ohq[:], vidx[:], None,
                                            op0=OP.is_equal)
                    nc.sync.dma_start(hc_next[E:E + 10, cols], ohq[:])
            c_cur = c_new
            hsp = split_to

        # ---- final scalars ----
        red6 = hd.tile([128, 6], F32, tag="red6", name="red6")
        nc.vector.tensor_reduce(red6[:], acc[:], axis=AX.X, op=OP.add)
        ps_s = psp.tile([1, 6], F32, tag="ps", name="ps")
        nc.tensor.matmul(ps_s[:], ones[:], red6[:], start=True, stop=True)
        ssb = hd.tile([1, 6], F32, tag="ssb", name="ssb")
        nc.vector.tensor_copy(ssb[:], ps_s[:])
        nc.sync.dma_start(scal_o.ap(), ssb[:])

        # ---- output DMAs (head-land -> [BP, k] row-major) ----
        def out_ap(handle, k):
            return bass.AP(tensor=handle, offset=0,
                           ap=[[k, 128], [128 * k, NT], [1, k]])

        nc.sync.dma_start(out_ap(nodes_o, 10), nodes_a[:])
        nc.sync.dma_start(out_ap(utt_o, MAX_LEN), utt_a[:])
        nc.sync.dma_start(out_ap(prop_o, NUM_ITEMS), prop_a[:])
        nc.sync.dma_start(
            bass.AP(tensor=aterm_o, offset=0, ap=[[1, 128], [128, NT]]),
            aterm_a[:])

    _split_excess_sync(nc)
    return nc


# ---------------------------------------------------------------- host prep
def _rng_draws():
    if "rng" in _CACHE:
        return _CACHE["rng"]
    import jax
    import jax.numpy as jnp
    cpu = jax.devices('cpu')[0]
    with jax.default_device(cpu):
        rng = jax.random.key(1234)
        u = np.asarray(jax.random.uniform(jax.random.fold_in(rng, 0),
                                          (BTOT, 1), jnp.float32))
        raws = [np.asarray(jax.random.normal(jax.random.fold_in(rng, 100 + i),
                                             (BTOT, 10), jnp.float32))
                for i in range(MAX_LEN)]
        gu = [np.asarray(jax.random.gumbel(jax.random.fold_in(rng, 200 + i),
                                           (BTOT, 10), jnp.float32))
              for i in range(MAX_LEN)]
        gp = [np.asarray(jax.random.gumbel(jax.random.fold_in(rng, 300 + i),
                                           (BTOT, 6), jnp.float32))
              for i in range(NUM_ITEMS)]
    rn = []
    for r in raws:
        nmin = r.min(axis=1, keepdims=True)
        nmax = r.max(axis=1, keepdims=True)
        rn.append(((r - nmin) / (nmax - nmin)).astype(np.float32))
    _CACHE["rng"] = (u, rn, gu, gp)
    return _CACHE["rng"]


def _hl(x):
    """[BP, k] (or [BP]) -> head-land [128, NT, k] / [128, NT]."""
    if x.ndim == 1:
        return np.ascontiguousarray(x.reshape(NT, 128).T)
    return np.ascontiguousarray(
        x.reshape(NT, 128, x.shape[1]).transpose(1, 0, 2))


def _bf16_split(x):
    import ml_dtypes
    bf16 = ml_dtypes.bfloat16
    xh = x.astype(bf16)
    xl = (x - xh.astype(np.float32)).astype(bf16)
    return np.ascontiguousarray(xh), np.ascontiguousarray(xl)


def _prep_in_maps(pool, utility, m_prev, prev_proposal, params):
    f32 = np.float32
    P = {k: np.asarray(v, f32) for k, v in params.items()}
    toks = np.stack([np.asarray(t, np.int64)[:, j].astype(f32)
                     for t, jr in [(pool, range(3)), (utility, range(3)),
                                   (m_prev, range(6)), (prev_proposal, range(3))]
                     for j in jr])  # [15, BTOT]

    wmap = {
        "b4_ctx": np.ascontiguousarray((P["ctx_bih"] + P["ctx_bhh"]).reshape(4, E).T),
        "b4_m": np.ascontiguousarray((P["utt_bih"] + P["utt_bhh"]).reshape(4, E).T),
        "b4_pr": np.ascontiguousarray((P["prop_bih"] + P["prop_bhh"]).reshape(4, E).T),
        "b4_up": np.ascontiguousarray((P["up_bih"] + P["up_bhh"]).reshape(4, E).T),
        "cwT": np.ascontiguousarray(P["comb_W"].T),
        "combb": P["comb_b"].reshape(E, 1),
        "headW": np.concatenate([P["term_W"].T] +
                                [P["pp_W"][i].T for i in range(NUM_ITEMS)], axis=1),
        "headb": np.broadcast_to(
            np.concatenate([P["term_b"]] + [P["pp_b"][i] for i in range(NUM_ITEMS)]),
            (128, 19)).copy(),
        "uhW": np.ascontiguousarray(P["up_h1_W"].T),
        "uhb": np.broadcast_to(P["up_h1_b"], (128, 10)).copy(),
    }
    wmap = {k: np.ascontiguousarray(v, dtype=f32) for k, v in wmap.items()}
    for key, xwm, hwm in [
            ("ctx", P["emb_ctx"] @ P["ctx_Wih"].T, P["ctx_Whh"].T),
            ("m", P["emb_utt"] @ P["utt_Wih"].T, P["utt_Whh"].T),
            ("pr", P["emb_ctx"] @ P["prop_Wih"].T, P["prop_Whh"].T),
            ("up", P["up_emb"] @ P["up_Wih"].T, P["up_Whh"].T)]:
        xh, xl = _bf16_split(np.ascontiguousarray(xwm, dtype=f32))
        hh, hl = _bf16_split(np.ascontiguousarray(hwm, dtype=f32))
        wmap[f"xwh_{key}"], wmap[f"xwl_{key}"] = xh, xl
        wmap[f"hwh_{key}"], wmap[f"hwl_{key}"] = hh, hl
        wmap[f"wch_{key}"] = np.concatenate([hh, xh], axis=0)
        wmap[f"wcl_{key}"] = np.concatenate([hl, xl], axis=0)

    u, rn, gu, gp = _rng_draws()
    in_maps = []
    for c in range(NCORES):
        sl = slice(c * BP, (c + 1) * BP)
        m = dict(wmap)
        m["tok"] = np.ascontiguousarray(toks[:, sl])
        m["ubern"] = _hl(u[sl, 0])
        m["rnp"] = np.ascontiguousarray(
            np.stack([_hl(rn[i][sl]) for i in range(MAX_LEN)], axis=1))
        m["gumu"] = np.ascontiguousarray(
            np.stack([_hl(gu[i][sl]) for i in range(MAX_LEN)], axis=1))
        m["gump"] = np.ascontiguousarray(
            np.stack([_hl(gp[i][sl]) for i in range(NUM_ITEMS)], axis=1))
        in_maps.append(m)
    return in_maps


def _get_nc():
    if "nc" not in _CACHE:
        _CACHE["nc"] = _build_program()
    return _CACHE["nc"]


def _run(in_maps, trace=False, trace_kwargs=None):
    from concourse.bass_utils import run_bass_kernel_spmd
    nc = _get_nc()
    kw = {}
    if trace:
        kw["trace"] = True
        if trace_kwargs:
            kw["trace_kwargs"] = trace_kwargs
    return run_bass_kernel_spmd(nc, in_maps, core_ids=list(range(NCORES)), **kw)


def _assemble(results):
    nodes = np.concatenate([r["nodes_o"] for r in results], axis=0)
    a_term = np.concatenate([r["aterm_o"] for r in results])[:, None]
    utterance = np.concatenate([r["utt_o"] for r in results], axis=0)
    proposal = np.concatenate([r["prop_o"] for r in results], axis=0)
    scal = np.stack([r["scal_o"][0] for r in results]).astype(np.float64)
    s_term, s_utt, s_prop, m_term, m_utt, m_prop = scal.sum(axis=0)
    entropy_loss = np.float32(0.05 * s_term + 0.001 * s_utt + 0.05 * s_prop)
    return (nodes.astype(np.float32), a_term.astype(np.uint8),
            utterance.astype(np.int32), proposal.astype(np.int32),
            entropy_loss, np.int32(round(m_term)), np.int32(round(m_utt)),
            MAX_LEN * BTOT, np.int32(round(m_prop)), NUM_ITEMS * BTOT)


def kernel(pool, utility, m_prev, prev_proposal, params):
    in_maps = _prep_in_maps(pool, utility, m_prev, prev_proposal, params)
    res = _run(in_maps, trace=False)
    return _assemble(res.results)


# revision 18
# speedup vs baseline: 1.8010x; 1.0835x over previous
"""Trainium2 Bass kernel for nn_AgentModel (negotiation agent forward pass).

Self-contained: takes FULL inputs, shards batch across 8 NeuronCores (pure
data parallel), runs one fp32 Bass/Tile program per core, gathers outputs.

Numerical strategy: all jax.random draws in the reference are input-
independent (uniform/normal/gumbel with fixed keys+shapes), so they are
precomputed on host with jax-CPU and fed to the device. All sampling
decisions (categorical argmax, greedy argmax, bernoulli compare) are made
from exact fp32 arithmetic on shifted logits -- the coarse ACT exp/ln LUTs
only touch output values (nodes/entropy) whose tolerances are loose.
"""
import numpy as np

E = 100
EPS = 1e-8
CORR = 0.1
MAX_LEN = 6
NUM_ITEMS = 3
NCORES = 8
BTOT = 32768
BP = BTOT // NCORES          # 4096 per core
NT = BP // 128               # 32 batch tiles of 128
NH = NT // 2                 # 16 tiles per half
BH = BP // 2                 # 2048 per half
NQT = NT // 4                # 8 tiles per quarter
BQ = BP // 4                 # 1024 per quarter
CH = 512                     # matmul N chunk (fp32 limit / PSUM bank)
NCH = BP // CH               # 8 chunks
BIG = 1e9

_CACHE = {}


# ---------------------------------------------------------------- sync fixup
def _split_excess_sync(nc, wait_limit=1, update_limit=1):
    """walrus rejects >1 sync wait/update per instruction in this toolchain;
    split extras onto adjacent same-engine NoOps (engines execute in order)."""
    import concourse.mybir as mybir
    n = 0
    for f in nc.m.functions:
        for bb in f.blocks:
            new_insts = []
            for ins in bb.instructions:
                si = ins.sync_info
                waits = list(si.on_wait) if si and si.on_wait else []
                updates = list(si.on_update) if si and si.on_update else []
                pre, post = [], []
                while len(waits) > wait_limit:
                    chunk, waits = waits[:wait_limit], waits[wait_limit:]
                    n += 1
                    pre.append(mybir.InstNoOp(
                        name=f"{ins.name}-ws{len(pre)}", engine=ins.engine,
                        ins=[], outs=[],
                        sync_info=mybir.SyncInfo(on_wait=chunk, on_update=[])))
                while len(updates) > update_limit:
                    chunk, updates = updates[:update_limit], updates[update_limit:]
                    n += 1
                    post.append(mybir.InstNoOp(
                        name=f"{ins.name}-us{len(post)}", engine=ins.engine,
                        ins=[], outs=[],
                        sync_info=mybir.SyncInfo(on_wait=[], on_update=chunk)))
                if si is not None:
                    si.on_wait = waits
                    si.on_update = updates
                new_insts.extend(pre)
                new_insts.append(ins)
                new_insts.extend(post)
            if len(new_insts) != len(bb.instructions):
                bb.instructions[:] = new_insts
    return n


# ---------------------------------------------------------------- program
def _build_program():
    import concourse.bass as bass
    import concourse.tile as tile
    from concourse import mybir
    from contextlib import ExitStack

    F32 = mybir.dt.float32
    I32 = mybir.dt.int32
    U8 = mybir.dt.uint8
    AF = mybir.ActivationFunctionType
    OP = mybir.AluOpType
    AX = mybir.AxisListType

    nc = bass.Bass("TRN2", target_bir_lowering=False, debug=False,
                   num_devices=NCORES)

    def dram_in(name, shape, dt=F32):
        return nc.dram_tensor(name, shape, dt, kind="ExternalInput")

    def dram_out(name, shape, dt=F32):
        return nc.dram_tensor(name, shape, dt, kind="ExternalOutput")

    tok_d = dram_in("tok", [15, BP])
    ubern_d = dram_in("ubern", [128, NT])
    rnp_d = dram_in("rnp", [128, MAX_LEN, NT, 10])
    gumu_d = dram_in("gumu", [128, MAX_LEN, NT, 10])
    gump_d = dram_in("gump", [128, NUM_ITEMS, NT, 6])
    BF16 = mybir.dt.bfloat16
    xw_d = {k: [dram_in(f"xw{p}_{k}", [v, 4 * E], BF16) for p in "hl"]
            for k, v in [("ctx", 6), ("m", 10), ("pr", 6), ("up", 10)]}
    hw_d = {k: [dram_in(f"hw{p}_{k}", [E, 4 * E], BF16) for p in "hl"]
            for k in ("ctx", "m", "pr", "up")}
    VOC = {"ctx": 6, "m": 10, "pr": 6, "up": 10}
    wc_d = {k: [dram_in(f"wc{p}_{k}", [E + VOC[k], 4 * E], BF16) for p in "hl"]
            for k in ("ctx", "m", "pr", "up")}
    b4_d = {k: dram_in(f"b4_{k}", [E, 4]) for k in ("ctx", "m", "pr", "up")}
    cwT_d = dram_in("cwT", [300, E])
    combb_d = dram_in("combb", [E, 1])
    headW_d = dram_in("headW", [E, 19])
    headb_d = dram_in("headb", [128, 19])
    uhW_d = dram_in("uhW", [E, 10])
    uhb_d = dram_in("uhb", [128, 10])

    nodes_o = dram_out("nodes_o", [BP, 10])
    aterm_o = dram_out("aterm_o", [BP], U8)
    utt_o = dram_out("utt_o", [BP, MAX_LEN], I32)
    prop_o = dram_out("prop_o", [BP, NUM_ITEMS], I32)
    scal_o = dram_out("scal_o", [1, 6])

    def row_bcast_ap(handle, row, nrows, ncols):
        """DRAM [R, C] row -> AP broadcasting that row across nrows partitions."""
        return bass.AP(tensor=handle, offset=row * ncols,
                       ap=[[0, nrows], [1, ncols]])

    def bc(ap3, n):
        """[128, X] AP -> [128, X, n] via trailing 0-stride broadcast."""
        return ap3.to_broadcast(list(ap3.shape) + [n])

    def bcmid(ap2, n):
        """[128, X] AP -> [128, n, X] via middle 0-stride broadcast."""
        return bass.AP(tensor=ap2.tensor, offset=ap2.offset,
                       ap=[list(ap2.ap[0]), [0, n], list(ap2.ap[1])])

    ctx = ExitStack()
    with tile.TileContext(nc) as tc, ctx:
        wsb = ctx.enter_context(tc.tile_pool(name="wsb", bufs=1))
        res = ctx.enter_context(tc.tile_pool(name="res", bufs=1))
        ohp = ctx.enter_context(tc.tile_pool(name="ohp", bufs=1))
        hp = ctx.enter_context(tc.tile_pool(name="hp", bufs=2))
        cp = ctx.enter_context(tc.tile_pool(name="cp", bufs=2))
        go = ctx.enter_context(tc.tile_pool(name="go", bufs=6))
        hd = ctx.enter_context(tc.tile_pool(name="hd", bufs=18))
        rgp = ctx.enter_context(tc.tile_pool(name="rgp", bufs=2))
        hd19 = ctx.enter_context(tc.tile_pool(name="hd19", bufs=2))
        psp = ctx.enter_context(tc.tile_pool(name="psp", bufs=8, space="PSUM"))
        drp = ctx.enter_context(tc.tile_pool(name="drp", bufs=4, space="DRAM"))

        # ---- load weights ----
        def wload(dram, shape, tag):
            t = wsb.tile(shape, F32, tag=tag, name=tag)
            nc.sync.dma_start(t[:], dram.ap())
            return t

        def wload2(pair, shape, tag):
            out = []
            for p, dram in zip("hl", pair):
                t = wsb.tile(shape, BF16, tag=f"{tag}{p}", name=f"{tag}{p}")
                nc.sync.dma_start(t[:], dram.ap())
                out.append(t)
            return out

        xw = {k: wload2(d, [d[0].shape[0], 4 * E], f"xw{k}") for k, d in xw_d.items()}
        hw = {k: wload2(d, [E, 4 * E], f"hw{k}") for k, d in hw_d.items()}
        wc = {k: wload2(d, [E + VOC[k], 4 * E], f"wc{k}") for k, d in wc_d.items()}
        b4 = {k: wload(d, [E, 4], f"b4{k}") for k, d in b4_d.items()}
        cwT = [wsb.tile([E, E], F32, tag=f"cwT{i}", name=f"cwT{i}") for i in range(3)]
        for i in range(3):
            nc.sync.dma_start(cwT[i][:], cwT_d.ap()[i * E:(i + 1) * E, :])
        combb = wload(combb_d, [E, 1], "combb")
        headW = wload(headW_d, [E, 19], "headW")
        headb = wload(headb_d, [128, 19], "headb")
        uhW = wload(uhW_d, [E, 10], "uhW")
        uhb = wload(uhb_d, [128, 10], "uhb")

        # ---- resident data ----
        ubern = res.tile([128, NT], F32, tag="ubern", name="ubern")
        nc.sync.dma_start(ubern[:], ubern_d.ap())
        gump = res.tile([128, NUM_ITEMS, NT, 6], F32, tag="gump", name="gump")
        nc.sync.dma_start(gump[:], gump_d.ap())

        # iota over class dim [128, NH, 10] and per-partition vocab index [10,1]
        iotaI = res.tile([128, NH, 10], I32, tag="iotaI", name="iotaI")
        nc.gpsimd.iota(iotaI[:], pattern=[[0, NH], [1, 10]], base=0,
                       channel_multiplier=0)
        iotaC = res.tile([128, NH, 10], F32, tag="iotaC", name="iotaC")
        nc.vector.tensor_copy(iotaC[:], iotaI[:])
        vidxI = res.tile([10, 1], I32, tag="vidxI", name="vidxI")
        nc.gpsimd.iota(vidxI[:], pattern=[[0, 1]], base=0, channel_multiplier=1)
        vidx = res.tile([10, 1], F32, tag="vidx", name="vidx")
        nc.vector.tensor_copy(vidx[:], vidxI[:])

        ones = res.tile([128, 1], F32, tag="ones", name="ones")
        nc.vector.memset(ones[:], 1.0)

        # accumulators: rows 0..5 = S_term, S_utt, S_prop, m_term, m_utt, m_prop
        acc = res.tile([128, 6, NT], F32, tag="acc", name="acc")
        nc.vector.memset(acc[:], 0.0)
        nodes_a = res.tile([128, NT, 10], F32, tag="nodes_a", name="nodes_a")
        utt_a = res.tile([128, NT, MAX_LEN], I32, tag="utt_a", name="utt_a")
        prop_a = res.tile([128, NT, NUM_ITEMS], I32, tag="prop_a", name="prop_a")
        aterm_a = res.tile([128, NT], U8, tag="aterm_a", name="aterm_a")

        comb_acc = res.tile([E, BP], F32, tag="comb_acc", name="comb_acc")
        h_t = res.tile([E, BP], F32, tag="h_t", name="h_t")
        hcp = ctx.enter_context(tc.tile_pool(name="hcp", bufs=2))
        ohq_p = ctx.enter_context(tc.tile_pool(name="ohq_p", bufs=2))
        hlp = ctx.enter_context(tc.tile_pool(name="hlp", bufs=2))

        # ---- one-hot build ----
        def build_onehot(row, vocab):
            # step-0 onehot at base partition 0 (standalone x-side matmul)
            oh = ohp.tile([10, BP], BF16, tag="oh", name="oh")
            nc.gpsimd.dma_start(oh[:vocab, :], row_bcast_ap(tok_d, row, vocab, BP))
            nc.vector.tensor_scalar(oh[:vocab, :], oh[:vocab, :], vidx[:vocab, :],
                                    None, op0=OP.is_equal)
            return oh

        def build_onehot_hc(row, vocab, hc):
            # compute-engine partition ranges must be 32-aligned, so build the
            # onehot at base partition 0 and DMA it into rows E..E+vocab
            oh = ohp.tile([10, BP], BF16, tag="oh", name="oh")
            nc.gpsimd.dma_start(oh[:vocab, :], row_bcast_ap(tok_d, row, vocab, BP))
            nc.vector.tensor_scalar(oh[:vocab, :], oh[:vocab, :],
                                    vidx[:vocab, :], None, op0=OP.is_equal)
            nc.sync.dma_start(hc[E:E + vocab, :], oh[:vocab, :])

        # ---- LSTM step (feature-land), cols = chunk range ----
        def lstm_chunks(xw_t, vocab, wc_t, hw_t, b4_t, oh, h_prev, c_prev,
                        h_new, c_new, chunks, split_to=None, grp_sz=2):
            # h_prev: None (h=0, oh = standalone onehot tile) or (hc, hl) where
            # hc = [hh; onehot] combined bf16 tile; c_prev None => c=0
            xwh, xwl = xw_t
            wch, wcl = wc_t
            hwh, hwl = hw_t
            KC = E + vocab
            ch_list = list(chunks)
            for k0 in range(0, len(ch_list), grp_sz):
                grp = ch_list[k0:k0 + grp_sz]
                pst = {(g, k): psp.tile([E, CH], F32, tag="ps", name="ps")
                       for g in range(4) for k in grp}
                if h_prev is None:
                    passes = [(xwh[:vocab, :], oh[:vocab, :], True),
                              (xwl[:vocab, :], oh[:vocab, :], False)]
                else:
                    hc, hl = h_prev
                    passes = [(wch[:KC, :], hc[:KC, :], True),
                              (wcl[:KC, :], hc[:KC, :], False),
                              (hwh[:, :], hl[:, :], False),
                              (hwl[:, :], hl[:, :], False)]
                np_ = len(passes)
                # weight-reuse: same stationary operand streams both chunks
                for g in range(4):
                    gs = slice(g * E, (g + 1) * E)
                    for pi, (w, rhs_t, st) in enumerate(passes):
                        wv = w[:, gs]
                        for k in grp:
                            cols = slice(k * CH, (k + 1) * CH)
                            nc.tensor.matmul(pst[(g, k)][:], wv,
                                             rhs_t[:, cols],
                                             start=st, stop=pi == np_ - 1,
                                             skip_group_check=True)
                for k in grp:
                    cols = slice(k * CH, (k + 1) * CH)
                    ig = go.tile([E, CH], F32, tag="go", name="go")
                    nc.scalar.activation(ig[:], pst[(0, k)][:], AF.Sigmoid,
                                         bias=b4_t[:, 0:1])
                    og = go.tile([E, CH], F32, tag="go", name="go")
                    nc.scalar.activation(og[:], pst[(3, k)][:], AF.Sigmoid,
                                         bias=b4_t[:, 3:4])
                    if c_prev is None:
                        gg = go.tile([E, CH], F32, tag="go", name="go")
                        nc.scalar.activation(gg[:], pst[(2, k)][:], AF.Tanh,
                                             bias=b4_t[:, 2:3])
                        nc.vector.tensor_mul(c_new[:, cols], ig[:], gg[:])
                    else:
                        fg = go.tile([E, CH], F32, tag="go", name="go")
                        nc.scalar.activation(fg[:], pst[(1, k)][:], AF.Sigmoid,
                                             bias=b4_t[:, 1:2])
                        gg = go.tile([E, CH], F32, tag="go", name="go")
                        nc.scalar.activation(gg[:], pst[(2, k)][:], AF.Tanh,
                                             bias=b4_t[:, 2:3])
                        t1 = go.tile([E, CH], F32, tag="go", name="go")
                        nc.vector.tensor_mul(t1[:], ig[:], gg[:])
                        t2 = go.tile([E, CH], F32, tag="go", name="go")
                        nc.vector.tensor_mul(t2[:], fg[:], c_prev[:, cols])
                        nc.vector.tensor_add(c_new[:, cols], t1[:], t2[:])
                    th = go.tile([E, CH], F32, tag="go", name="go")
                    nc.scalar.activation(th[:], c_new[:, cols], AF.Tanh)
                    nc.vector.tensor_mul(h_new[:, cols], og[:], th[:])
                    if split_to is not None:
                        shc, shl = split_to
                        nc.scalar.copy(shc[:E, cols], h_new[:, cols])
                        nc.vector.tensor_tensor(shl[:, cols], h_new[:, cols],
                                                shc[:E, cols], op=OP.subtract)

        # ---- encoder ----
        def encoder(key, vocab, row0, steps):
            hsp = c_prev = None
            oh = build_onehot(row0, vocab)
            for s in range(steps):
                h_new = hp.tile([E, BP], F32, tag="h", name="h")
                c_new = cp.tile([E, BP], F32, tag="c", name="c")
                split_to = None
                if s + 1 < steps:
                    hc_next = hcp.tile([112, BP], BF16, tag="hc", name="hc")
                    build_onehot_hc(row0 + s + 1, vocab, hc_next)
                    split_to = (hc_next,
                                hlp.tile([E, BP], BF16, tag="hl", name="hl"))
                lstm_chunks(xw[key], vocab, wc[key], hw[key], b4[key], oh, hsp,
                            c_prev, h_new, c_new, range(NCH), split_to)
                hsp, c_prev = split_to, c_new
                oh = None
            return h_new

        def comb_add(enc_idx, h_enc):
            for k in range(NCH):
                cols = slice(k * CH, (k + 1) * CH)
                ps = psp.tile([128, CH], F32, tag="ps", name="ps")
                nc.tensor.matmul(ps[:E, :], cwT[enc_idx][:], h_enc[:, cols],
                                 start=True, stop=True)
                if enc_idx == 0:
                    nc.vector.tensor_copy(comb_acc[:, cols], ps[:E, :])
                else:
                    nc.vector.tensor_add(comb_acc[:, cols], comb_acc[:, cols],
                                         ps[:E, :])

        c_h = encoder("ctx", 6, 0, 6)
        comb_add(0, c_h)
        m_h = encoder("m", 10, 6, 6)
        comb_add(1, m_h)
        p_h = encoder("pr", 6, 12, 3)
        comb_add(2, p_h)
        nc.scalar.activation(h_t[:], comb_acc[:], AF.Relu, bias=combb[:, 0:1])

        # ---- shared softmax/sample in head-land ----
        def softmax_sample(l2, C, nt, gum_ap, hsl, nodes_col, tok_slice,
                           ent_row, match_row):
            """l2: [128, nt, C] logits AP. Returns sampled index tile [128, nt]."""
            smax = hd.tile([128, nt], F32, tag="hs", name="hs")
            nc.vector.tensor_reduce(smax[:], l2, axis=AX.X, op=OP.max)
            sh = hd.tile([128, nt, 10], F32, tag="hd", name="hd")[:, :, :C]
            nc.vector.tensor_tensor(sh, l2, bc(smax[:], C), op=OP.subtract)
            ex = hd.tile([128, nt, 10], F32, tag="hd", name="hd")[:, :, :C]
            nc.scalar.activation(ex, sh, AF.Exp)
            S = hd.tile([128, nt], F32, tag="hs", name="hs")
            nc.vector.tensor_reduce(S[:], ex, axis=AX.X, op=OP.add)
            rec = hd.tile([128, nt], F32, tag="hs", name="hs")
            nc.vector.reciprocal(rec[:], S[:])
            probs = hd.tile([128, nt, 10], F32, tag="hd", name="hd")[:, :, :C]
            nc.vector.tensor_tensor(probs, ex, bc(rec[:], C), op=OP.mult)
            # entropy contribution sum_c (p+EPS)ln(p+EPS)
            q = hd.tile([128, nt, 10], F32, tag="hd", name="hd")[:, :, :C]
            nc.vector.tensor_single_scalar(q, probs, EPS, op=OP.add)
            lnq = hd.tile([128, nt, 10], F32, tag="hd", name="hd")[:, :, :C]
            nc.scalar.activation(lnq, q, AF.Ln)
            ql = hd.tile([128, nt, 10], F32, tag="hd", name="hd")[:, :, :C]
            nc.vector.tensor_mul(ql, q, lnq)
            ctr = hd.tile([128, nt], F32, tag="hs", name="hs")
            nc.vector.tensor_reduce(ctr[:], ql, axis=AX.X, op=OP.add)
            nc.vector.tensor_add(acc[:, ent_row, hsl], acc[:, ent_row, hsl],
                                 ctr[:])
            # categorical sample: argmax(sh + gumbel) (lnS shift cancels)
            v = hd.tile([128, nt, 10], F32, tag="hd", name="hd")[:, :, :C]
            nc.vector.tensor_tensor(v, sh, gum_ap, op=OP.add)
            vmax = hd.tile([128, nt], F32, tag="hs", name="hs")
            nc.vector.tensor_reduce(vmax[:], v, axis=AX.X, op=OP.max)
            veq = hd.tile([128, nt, 10], F32, tag="hd", name="hd")[:, :, :C]
            nc.vector.tensor_tensor(veq, v, bc(vmax[:], C), op=OP.is_equal)
            msk = hd.tile([128, nt, 10], F32, tag="hd", name="hd")[:, :, :C]
            nc.vector.tensor_mul(msk, veq, iotaC[:, :nt, :C])
            pen = hd.tile([128, nt, 10], F32, tag="hd", name="hd")[:, :, :C]
            nc.vector.tensor_scalar(pen, veq, -BIG, BIG, op0=OP.mult, op1=OP.add)
            nc.vector.tensor_add(msk, msk, pen)
            af = hd.tile([128, nt], F32, tag="hs", name="hs")
            nc.vector.tensor_reduce(af[:], msk, axis=AX.X, op=OP.min)
            nc.vector.tensor_copy(tok_slice, af[:])        # cast f32 -> i32
            # chosen prob -> nodes; greedy match via shifted-logit == 0
            aeq = hd.tile([128, nt, 10], F32, tag="hd", name="hd")[:, :, :C]
            nc.vector.tensor_tensor(aeq, iotaC[:, :nt, :C], bc(af[:], C),
                                    op=OP.is_equal)
            pa = hd.tile([128, nt, 10], F32, tag="hd", name="hd")[:, :, :C]
            nc.vector.tensor_mul(pa, probs, aeq)
            pa1 = hd.tile([128, nt], F32, tag="hs", name="hs")
            nc.vector.tensor_reduce(pa1[:], pa, axis=AX.X, op=OP.add)
            nc.scalar.activation(nodes_a[:, hsl, nodes_col], pa1[:], AF.Ln)
            ssel = hd.tile([128, nt, 10], F32, tag="hd", name="hd")[:, :, :C]
            nc.vector.tensor_mul(ssel, sh, aeq)
            ss1 = hd.tile([128, nt], F32, tag="hs", name="hs")
            nc.vector.tensor_reduce(ss1[:], ssel, axis=AX.X, op=OP.add)
            mt = hd.tile([128, nt], F32, tag="hs", name="hs")
            nc.vector.tensor_single_scalar(mt[:], ss1[:], 0.0, op=OP.is_equal)
            nc.vector.tensor_add(acc[:, match_row, hsl], acc[:, match_row, hsl],
                                 mt[:])
            return af

        # ---- term + proposal heads ----
        for half in range(2):
            hsl = slice(half * NH, (half + 1) * NH)
            ps19 = psp.tile([128, NH, 19], F32, tag="ps", name="ps")
            for t16 in range(NH):
                t = half * NH + t16
                nc.tensor.matmul(ps19[:, t16, :], h_t[:, t * 128:(t + 1) * 128],
                                 headW[:], start=True, stop=True)
            lg19 = hd19.tile([128, NH, 19], F32, tag="hd19", name="hd19")
            nc.vector.tensor_tensor(lg19[:], ps19[:], bcmid(headb[:], NH),
                                    op=OP.add)
            # term policy
            tp = hd.tile([128, NH], F32, tag="hs", name="hs")
            nc.scalar.activation(tp[:], lg19[:, :, 0], AF.Sigmoid)
            au = hd.tile([128, NH], F32, tag="hs", name="hs")
            nc.vector.tensor_tensor(au[:], ubern[:, hsl], tp[:], op=OP.is_lt)
            nc.vector.tensor_copy(aterm_a[:, hsl], au[:])   # cast -> u8
            rg = hd.tile([128, NH], F32, tag="hs", name="hs")
            nc.vector.tensor_single_scalar(rg[:], tp[:], 0.5, op=OP.is_ge)
            mt = hd.tile([128, NH], F32, tag="hs", name="hs")
            nc.vector.tensor_tensor(mt[:], rg[:], au[:], op=OP.is_equal)
            nc.vector.tensor_add(acc[:, 3, hsl], acc[:, 3, hsl], mt[:])
            # g = a*p + (1-a)*(1-p), exact fp32 mirror of the reference
            aup = hd.tile([128, NH], F32, tag="hs", name="hs")
            nc.vector.tensor_mul(aup[:], au[:], tp[:])
            nau = hd.tile([128, NH], F32, tag="hs", name="hs")
            nc.vector.tensor_scalar(nau[:], au[:], -1.0, 1.0, op0=OP.mult,
                                    op1=OP.add)
            ntp = hd.tile([128, NH], F32, tag="hs", name="hs")
            nc.vector.tensor_scalar(ntp[:], tp[:], -1.0, 1.0, op0=OP.mult,
                                    op1=OP.add)
            t2 = hd.tile([128, NH], F32, tag="hs", name="hs")
            nc.vector.tensor_mul(t2[:], nau[:], ntp[:])
            gt = hd.tile([128, NH], F32, tag="hs", name="hs")
            nc.vector.tensor_add(gt[:], aup[:], t2[:])
            nc.scalar.activation(nodes_a[:, hsl, 0], gt[:], AF.Ln)
            q = hd.tile([128, NH], F32, tag="hs", name="hs")
            nc.vector.tensor_single_scalar(q[:], tp[:], EPS, op=OP.add)
            lnq = hd.tile([128, NH], F32, tag="hs", name="hs")
            nc.scalar.activation(lnq[:], q[:], AF.Ln)
            ql = hd.tile([128, NH], F32, tag="hs", name="hs")
            nc.vector.tensor_mul(ql[:], q[:], lnq[:])
            nc.vector.tensor_add(acc[:, 0, hsl], acc[:, 0, hsl], ql[:])
            # proposal heads
            for i in range(NUM_ITEMS):
                softmax_sample(lg19[:, :, 1 + 6 * i:7 + 6 * i], 6, NH,
                               gump[:, i, hsl, :], hsl, 7 + i,
                               prop_a[:, hsl, i], 2, 5)

        # ---- utterance decode (autoregressive, pipelined in quarters) ----
        hc0 = hcp.tile([112, BP], BF16, tag="hc", name="hc")
        oh0 = ohp.tile([10, BP], BF16, tag="oh", name="oh")
        nc.vector.memset(oh0[:], 0.0)
        nc.vector.memset(oh0[0:1, :], 1.0)
        nc.sync.dma_start(hc0[E:E + 10, :], oh0[:])
        h_tl = hlp.tile([E, BP], BF16, tag="hl", name="hl")
        nc.scalar.copy(hc0[:E, :], h_t[:])
        nc.vector.tensor_tensor(h_tl[:], h_t[:], hc0[:E, :], op=OP.subtract)
        hsp = (hc0, h_tl)
        c_cur = None
        for s in range(MAX_LEN):
            rnp_s = rgp.tile([128, NT, 10], F32, tag="rnp", name="rnp")
            nc.sync.dma_start(rnp_s[:], rnp_d.ap()[:, s, :, :])
            gumu_s = rgp.tile([128, NT, 10], F32, tag="gumu", name="gumu")
            nc.sync.dma_start(gumu_s[:], gumu_d.ap()[:, s, :, :])
            h_new = hp.tile([E, BP], F32, tag="h", name="h")
            c_new = cp.tile([E, BP], F32, tag="c", name="c")
            split_to = None
            if s + 1 < MAX_LEN:
                split_to = (hcp.tile([112, BP], BF16, tag="hc", name="hc"),
                            hlp.tile([E, BP], BF16, tag="hl", name="hl"))
            for q in range(4):
                lstm_chunks(xw["up"], 10, wc["up"], hw["up"], b4["up"], None,
                            hsp, c_cur, h_new, c_new,
                            range(q * 2, q * 2 + 2), split_to, grp_sz=1)
                qsl = slice(q * NQT, (q + 1) * NQT)
                ps10 = psp.tile([128, NQT, 10], F32, tag="ps", name="ps")
                for tq in range(NQT):
                    t = q * NQT + tq
                    nc.tensor.matmul(ps10[:, tq, :],
                                     h_new[:, t * 128:(t + 1) * 128],
                                     uhW[:], start=True, stop=True)
                lg0 = hd.tile([128, NQT, 10], F32, tag="hd", name="hd")
                nc.vector.tensor_tensor(lg0[:], ps10[:], bcmid(uhb[:], NQT),
                                        op=OP.add)
                # noise blend: l2 = 0.9*l + 0.1*(lmin + (lmax-lmin)*rn)
                rmn = hd.tile([128, NQT], F32, tag="hs", name="hs")
                nc.vector.tensor_reduce(rmn[:], lg0[:], axis=AX.X, op=OP.min)
                rmx = hd.tile([128, NQT], F32, tag="hs", name="hs")
                nc.vector.tensor_reduce(rmx[:], lg0[:], axis=AX.X, op=OP.max)
                d = hd.tile([128, NQT], F32, tag="hs", name="hs")
                nc.vector.tensor_tensor(d[:], rmx[:], rmn[:], op=OP.subtract)
                n1 = hd.tile([128, NQT, 10], F32, tag="hd", name="hd")
                nc.vector.tensor_tensor(n1[:], rnp_s[:, qsl, :], bc(d[:], 10),
                                        op=OP.mult)
                noise = hd.tile([128, NQT, 10], F32, tag="hd", name="hd")
                nc.vector.tensor_tensor(noise[:], n1[:], bc(rmn[:], 10),
                                        op=OP.add)
                ns = hd.tile([128, NQT, 10], F32, tag="hd", name="hd")
                nc.vector.tensor_single_scalar(ns[:], noise[:], CORR, op=OP.mult)
                l2 = hd.tile([128, NQT, 10], F32, tag="hd", name="hd")
                nc.vector.scalar_tensor_tensor(l2[:], lg0[:], 1.0 - CORR, ns[:],
                                               op0=OP.mult, op1=OP.add)
                af = softmax_sample(l2[:], 10, NQT, gumu_s[:, qsl, :], qsl,
                                    1 + s, utt_a[:, qsl, s], 1, 4)
                if s + 1 < MAX_LEN:
                    hc_next = split_to[0]
                    rt = drp.tile([BQ], F32, tag="rt", name="rt")
                    rbase = rt[:]
                    nc.sync.dma_start(
                        bass.AP(tensor=rbase.tensor, offset=rbase.offset,
                                ap=[[1, 128], [128, NQT]]), af[:])
                    cols = slice(q * BQ, (q + 1) * BQ)
                    ohq = ohq_p.tile([10, BQ], BF16, tag="ohq", name="ohq")
                    nc.gpsimd.dma_start(
                        ohq[:],
                        bass.AP(tensor=rbase.tensor, offset=rbase.offset,
                                ap=[[0, 10], [1, BQ]]))
                    nc.vector.tensor_scalar(ohq[:], ohq[:], vidx[:], None,
                                            op0=OP.is_equal)
                    nc.sync.dma_start(hc_next[E:E + 10, cols], ohq[:])
            c_cur = c_new
            hsp = split_to

        # ---- final scalars ----
        red6 = hd.tile([128, 6], F32, tag="red6", name="red6")
        nc.vector.tensor_reduce(red6[:], acc[:], axis=AX.X, op=OP.add)
        ps_s = psp.tile([1, 6], F32, tag="ps", name="ps")
        nc.tensor.matmul(ps_s[:], ones[:], red6[:], start=True, stop=True)
        ssb = hd.tile([1, 6], F32, tag="ssb", name="ssb")
        nc.vector.tensor_copy(ssb[:], ps_s[:])
        nc.sync.dma_start(scal_o.ap(), ssb[:])

        # ---- output DMAs (head-land -> [BP, k] row-major) ----
        def out_ap(handle, k):
            return bass.AP(tensor=handle, offset=0,
                           ap=[[k, 128], [128 * k, NT], [1, k]])

        nc.sync.dma_start(out_ap(nodes_o, 10), nodes_a[:])
        nc.sync.dma_start(out_ap(utt_o, MAX_LEN), utt_a[:])
        nc.sync.dma_start(out_ap(prop_o, NUM_ITEMS), prop_a[:])
        nc.sync.dma_start(
            bass.AP(tensor=aterm_o, offset=0, ap=[[1, 128], [128, NT]]),
            aterm_a[:])

    _split_excess_sync(nc)
    return nc


# ---------------------------------------------------------------- host prep
def _rng_draws():
    if "rng" in _CACHE:
        return _CACHE["rng"]
    import jax
    import jax.numpy as jnp
    cpu = jax.devices('cpu')[0]
    with jax.default_device(cpu):
        rng = jax.random.key(1234)
        u = np.asarray(jax.random.uniform(jax.random.fold_in(rng, 0),
                                          (BTOT, 1), jnp.float32))
        raws = [np.asarray(jax.random.normal(jax.random.fold_in(rng, 100 + i),
                                             (BTOT, 10), jnp.float32))
                for i in range(MAX_LEN)]
        gu = [np.asarray(jax.random.gumbel(jax.random.fold_in(rng, 200 + i),
                                           (BTOT, 10), jnp.float32))
              for i in range(MAX_LEN)]
        gp = [np.asarray(jax.random.gumbel(jax.random.fold_in(rng, 300 + i),
                                           (BTOT, 6), jnp.float32))
              for i in range(NUM_ITEMS)]
    rn = []
    for r in raws:
        nmin = r.min(axis=1, keepdims=True)
        nmax = r.max(axis=1, keepdims=True)
        rn.append(((r - nmin) / (nmax - nmin)).astype(np.float32))
    _CACHE["rng"] = (u, rn, gu, gp)
    return _CACHE["rng"]


def _hl(x):
    """[BP, k] (or [BP]) -> head-land [128, NT, k] / [128, NT]."""
    if x.ndim == 1:
        return np.ascontiguousarray(x.reshape(NT, 128).T)
    return np.ascontiguousarray(
        x.reshape(NT, 128, x.shape[1]).transpose(1, 0, 2))


def _bf16_split(x):
    import ml_dtypes
    bf16 = ml_dtypes.bfloat16
    xh = x.astype(bf16)
    xl = (x - xh.astype(np.float32)).astype(bf16)
    return np.ascontiguousarray(xh), np.ascontiguousarray(xl)


def _prep_in_maps(pool, utility, m_prev, prev_proposal, params):
    f32 = np.float32
    P = {k: np.asarray(v, f32) for k, v in params.items()}
    toks = np.stack([np.asarray(t, np.int64)[:, j].astype(f32)
                     for t, jr in [(pool, range(3)), (utility, range(3)),
                                   (m_prev, range(6)), (prev_proposal, range(3))]
                     for j in jr])  # [15, BTOT]

    wmap = {
        "b4_ctx": np.ascontiguousarray((P["ctx_bih"] + P["ctx_bhh"]).reshape(4, E).T),
        "b4_m": np.ascontiguousarray((P["utt_bih"] + P["utt_bhh"]).reshape(4, E).T),
        "b4_pr": np.ascontiguousarray((P["prop_bih"] + P["prop_bhh"]).reshape(4, E).T),
        "b4_up": np.ascontiguousarray((P["up_bih"] + P["up_bhh"]).reshape(4, E).T),
        "cwT": np.ascontiguousarray(P["comb_W"].T),
        "combb": P["comb_b"].reshape(E, 1),
        "headW": np.concatenate([P["term_W"].T] +
                                [P["pp_W"][i].T for i in range(NUM_ITEMS)], axis=1),
        "headb": np.broadcast_to(
            np.concatenate([P["term_b"]] + [P["pp_b"][i] for i in range(NUM_ITEMS)]),
            (128, 19)).copy(),
        "uhW": np.ascontiguousarray(P["up_h1_W"].T),
        "uhb": np.broadcast_to(P["up_h1_b"], (128, 10)).copy(),
    }
    wmap = {k: np.ascontiguousarray(v, dtype=f32) for k, v in wmap.items()}
    for key, xwm, hwm in [
            ("ctx", P["emb_ctx"] @ P["ctx_Wih"].T, P["ctx_Whh"].T),
            ("m", P["emb_utt"] @ P["utt_Wih"].T, P["utt_Whh"].T),
            ("pr", P["emb_ctx"] @ P["prop_Wih"].T, P["prop_Whh"].T),
            ("up", P["up_emb"] @ P["up_Wih"].T, P["up_Whh"].T)]:
        xh, xl = _bf16_split(np.ascontiguousarray(xwm, dtype=f32))
        hh, hl = _bf16_split(np.ascontiguousarray(hwm, dtype=f32))
        wmap[f"xwh_{key}"], wmap[f"xwl_{key}"] = xh, xl
        wmap[f"hwh_{key}"], wmap[f"hwl_{key}"] = hh, hl
        wmap[f"wch_{key}"] = np.concatenate([hh, xh], axis=0)
        wmap[f"wcl_{key}"] = np.concatenate([hl, xl], axis=0)

    u, rn, gu, gp = _rng_draws()
    in_maps = []
    for c in range(NCORES):
        sl = slice(c * BP, (c + 1) * BP)
        m = dict(wmap)
        m["tok"] = np.ascontiguousarray(toks[:, sl])
        m["ubern"] = _hl(u[sl, 0])
        m["rnp"] = np.ascontiguousarray(
            np.stack([_hl(rn[i][sl]) for i in range(MAX_LEN)], axis=1))
        m["gumu"] = np.ascontiguousarray(
            np.stack([_hl(gu[i][sl]) for i in range(MAX_LEN)], axis=1))
        m["gump"] = np.ascontiguousarray(
            np.stack([_hl(gp[i][sl]) for i in range(NUM_ITEMS)], axis=1))
        in_maps.append(m)
    return in_maps


def _get_nc():
    if "nc" not in _CACHE:
        _CACHE["nc"] = _build_program()
    return _CACHE["nc"]


def _run(in_maps, trace=False, trace_kwargs=None):
    from concourse.bass_utils import run_bass_kernel_spmd
    nc = _get_nc()
    kw = {}
    if trace:
        kw["trace"] = True
        if trace_kwargs:
            kw["trace_kwargs"] = trace_kwargs
    return run_bass_kernel_spmd(nc, in_maps, core_ids=list(range(NCORES)), **kw)


def _assemble(results):
    nodes = np.concatenate([r["nodes_o"] for r in results], axis=0)
    a_term = np.concatenate([r["aterm_o"] for r in results])[:, None]
    utterance = np.concatenate([r["utt_o"] for r in results], axis=0)
    proposal = np.concatenate([r["prop_o"] for r in results], axis=0)
    scal = np.stack([r["scal_o"][0] for r in results]).astype(np.float64)
    s_term, s_utt, s_prop, m_term, m_utt, m_prop = scal.sum(axis=0)
    entropy_loss = np.float32(0.05 * s_term + 0.001 * s_utt + 0.05 * s_prop)
    return (nodes.astype(np.float32), a_term.astype(np.uint8),
            utterance.astype(np.int32), proposal.astype(np.int32),
            entropy_loss, np.int32(round(m_term)), np.int32(round(m_utt)),
            MAX_LEN * BTOT, np.int32(round(m_prop)), NUM_ITEMS * BTOT)


def kernel(pool, utility, m_prev, prev_proposal, params):
    in_maps = _prep_in_maps(pool, utility, m_prev, prev_proposal, params)
    res = _run(in_maps, trace=False)
    return _assemble(res.results)


# revision 22
# speedup vs baseline: 1.8075x; 1.0036x over previous
"""Trainium2 Bass kernel for nn_AgentModel (negotiation agent forward pass).

Self-contained: takes FULL inputs, shards batch across 8 NeuronCores (pure
data parallel), runs one fp32 Bass/Tile program per core, gathers outputs.

Numerical strategy: all jax.random draws in the reference are input-
independent (uniform/normal/gumbel with fixed keys+shapes), so they are
precomputed on host with jax-CPU and fed to the device. All sampling
decisions (categorical argmax, greedy argmax, bernoulli compare) are made
from exact fp32 arithmetic on shifted logits -- the coarse ACT exp/ln LUTs
only touch output values (nodes/entropy) whose tolerances are loose.
"""
import numpy as np

E = 100
EPS = 1e-8
CORR = 0.1
MAX_LEN = 6
NUM_ITEMS = 3
NCORES = 8
BTOT = 32768
BP = BTOT // NCORES          # 4096 per core
NT = BP // 128               # 32 batch tiles of 128
NH = NT // 2                 # 16 tiles per half
BH = BP // 2                 # 2048 per half
NQT = NT // 4                # 8 tiles per quarter
BQ = BP // 4                 # 1024 per quarter
CH = 512                     # matmul N chunk (fp32 limit / PSUM bank)
NCH = BP // CH               # 8 chunks
BIG = 1e9

_CACHE = {}


# ---------------------------------------------------------------- sync fixup
def _split_excess_sync(nc, wait_limit=1, update_limit=1):
    """walrus rejects >1 sync wait/update per instruction in this toolchain;
    split extras onto adjacent same-engine NoOps (engines execute in order)."""
    import concourse.mybir as mybir
    n = 0
    for f in nc.m.functions:
        for bb in f.blocks:
            new_insts = []
            for ins in bb.instructions:
                si = ins.sync_info
                waits = list(si.on_wait) if si and si.on_wait else []
                updates = list(si.on_update) if si and si.on_update else []
                pre, post = [], []
                while len(waits) > wait_limit:
                    chunk, waits = waits[:wait_limit], waits[wait_limit:]
                    n += 1
                    pre.append(mybir.InstNoOp(
                        name=f"{ins.name}-ws{len(pre)}", engine=ins.engine,
                        ins=[], outs=[],
                        sync_info=mybir.SyncInfo(on_wait=chunk, on_update=[])))
                while len(updates) > update_limit:
                    chunk, updates = updates[:update_limit], updates[update_limit:]
                    n += 1
                    post.append(mybir.InstNoOp(
                        name=f"{ins.name}-us{len(post)}", engine=ins.engine,
                        ins=[], outs=[],
                        sync_info=mybir.SyncInfo(on_wait=[], on_update=chunk)))
                if si is not None:
                    si.on_wait = waits
                    si.on_update = updates
                new_insts.extend(pre)
                new_insts.append(ins)
                new_insts.extend(post)
            if len(new_insts) != len(bb.instructions):
                bb.instructions[:] = new_insts
    return n


# ---------------------------------------------------------------- program
def _build_program():
    import concourse.bass as bass
    import concourse.tile as tile
    from concourse import mybir
    from contextlib import ExitStack

    F32 = mybir.dt.float32
    I32 = mybir.dt.int32
    U8 = mybir.dt.uint8
    AF = mybir.ActivationFunctionType
    OP = mybir.AluOpType
    AX = mybir.AxisListType

    nc = bass.Bass("TRN2", target_bir_lowering=False, debug=False,
                   num_devices=NCORES)

    def dram_in(name, shape, dt=F32):
        return nc.dram_tensor(name, shape, dt, kind="ExternalInput")

    def dram_out(name, shape, dt=F32):
        return nc.dram_tensor(name, shape, dt, kind="ExternalOutput")

    tok_d = dram_in("tok", [15, BP])
    ubern_d = dram_in("ubern", [128, NT])
    rnp_d = dram_in("rnp", [128, MAX_LEN, NT, 10])
    gumu_d = dram_in("gumu", [128, MAX_LEN, NT, 10])
    gump_d = dram_in("gump", [128, NUM_ITEMS, NT, 6])
    BF16 = mybir.dt.bfloat16
    xw_d = {k: [dram_in(f"xw{p}_{k}", [v, 4 * E], BF16) for p in "hl"]
            for k, v in [("ctx", 6), ("m", 10), ("pr", 6), ("up", 10)]}
    hw_d = {k: [dram_in(f"hw{p}_{k}", [E, 4 * E], BF16) for p in "hl"]
            for k in ("ctx", "m", "pr", "up")}
    VOC = {"ctx": 6, "m": 10, "pr": 6, "up": 10}
    wc_d = {k: [dram_in(f"wc{p}_{k}", [E + VOC[k], 4 * E], BF16) for p in "hl"]
            for k in ("ctx", "m", "pr", "up")}
    b4_d = {k: dram_in(f"b4_{k}", [E, 4]) for k in ("ctx", "m", "pr", "up")}
    prop_id_d = dram_in("prop_id", [BP])
    poh_d = dram_in("poh", [6, 3 * 216], mybir.dt.bfloat16)
    cwT_d = dram_in("cwT", [300, E])
    combb_d = dram_in("combb", [E, 1])
    headW_d = dram_in("headW", [E, 19])
    headb_d = dram_in("headb", [128, 19])
    uhW_d = dram_in("uhW", [E, 10])
    uhb_d = dram_in("uhb", [128, 10])

    nodes_o = dram_out("nodes_o", [BP, 10])
    aterm_o = dram_out("aterm_o", [BP], U8)
    utt_o = dram_out("utt_o", [BP, MAX_LEN], I32)
    prop_o = dram_out("prop_o", [BP, NUM_ITEMS], I32)
    scal_o = dram_out("scal_o", [1, 6])

    def row_bcast_ap(handle, row, nrows, ncols):
        """DRAM [R, C] row -> AP broadcasting that row across nrows partitions."""
        return bass.AP(tensor=handle, offset=row * ncols,
                       ap=[[0, nrows], [1, ncols]])

    def bc(ap3, n):
        """[128, X] AP -> [128, X, n] via trailing 0-stride broadcast."""
        return ap3.to_broadcast(list(ap3.shape) + [n])

    def bcmid(ap2, n):
        """[128, X] AP -> [128, n, X] via middle 0-stride broadcast."""
        return bass.AP(tensor=ap2.tensor, offset=ap2.offset,
                       ap=[list(ap2.ap[0]), [0, n], list(ap2.ap[1])])

    ctx = ExitStack()
    with tile.TileContext(nc) as tc, ctx:
        wsb = ctx.enter_context(tc.tile_pool(name="wsb", bufs=1))
        res = ctx.enter_context(tc.tile_pool(name="res", bufs=1))
        ohp = ctx.enter_context(tc.tile_pool(name="ohp", bufs=1))
        hp = ctx.enter_context(tc.tile_pool(name="hp", bufs=2))
        cp = ctx.enter_context(tc.tile_pool(name="cp", bufs=2))
        go = ctx.enter_context(tc.tile_pool(name="go", bufs=6))
        hd = ctx.enter_context(tc.tile_pool(name="hd", bufs=12))
        rgp = ctx.enter_context(tc.tile_pool(name="rgp", bufs=2))
        hd19 = ctx.enter_context(tc.tile_pool(name="hd19", bufs=2))
        psp = ctx.enter_context(tc.tile_pool(name="psp", bufs=8, space="PSUM"))
        drp = ctx.enter_context(tc.tile_pool(name="drp", bufs=4, space="DRAM"))

        # ---- load weights ----
        def wload(dram, shape, tag):
            t = wsb.tile(shape, F32, tag=tag, name=tag)
            nc.sync.dma_start(t[:], dram.ap())
            return t

        def wload2(pair, shape, tag):
            out = []
            for p, dram in zip("hl", pair):
                t = wsb.tile(shape, BF16, tag=f"{tag}{p}", name=f"{tag}{p}")
                nc.sync.dma_start(t[:], dram.ap())
                out.append(t)
            return out

        xw = {k: wload2(d, [d[0].shape[0], 4 * E], f"xw{k}") for k, d in xw_d.items()}
        hw = {k: wload2(d, [E, 4 * E], f"hw{k}") for k, d in hw_d.items()}
        wc = {k: wload2(d, [E + VOC[k], 4 * E], f"wc{k}") for k, d in wc_d.items()}
        b4 = {k: wload(d, [E, 4], f"b4{k}") for k, d in b4_d.items()}
        cwT = [wsb.tile([E, E], F32, tag=f"cwT{i}", name=f"cwT{i}") for i in range(3)]
        for i in range(3):
            nc.sync.dma_start(cwT[i][:], cwT_d.ap()[i * E:(i + 1) * E, :])
        combb = wload(combb_d, [E, 1], "combb")
        headW = wload(headW_d, [E, 19], "headW")
        headb = wload(headb_d, [128, 19], "headb")
        uhW = wload(uhW_d, [E, 10], "uhW")
        uhb = wload(uhb_d, [128, 10], "uhb")

        # ---- resident data ----
        ubern = res.tile([128, NT], F32, tag="ubern", name="ubern")
        nc.sync.dma_start(ubern[:], ubern_d.ap())
        gump = res.tile([128, NUM_ITEMS, NT, 6], F32, tag="gump", name="gump")
        nc.sync.dma_start(gump[:], gump_d.ap())

        # iota over class dim [128, NH, 10] and per-partition vocab index [10,1]
        iotaI = res.tile([128, NH, 10], I32, tag="iotaI", name="iotaI")
        nc.gpsimd.iota(iotaI[:], pattern=[[0, NH], [1, 10]], base=0,
                       channel_multiplier=0)
        iotaC = res.tile([128, NH, 10], F32, tag="iotaC", name="iotaC")
        nc.vector.tensor_copy(iotaC[:], iotaI[:])
        vidxI = res.tile([10, 1], I32, tag="vidxI", name="vidxI")
        nc.gpsimd.iota(vidxI[:], pattern=[[0, 1]], base=0, channel_multiplier=1)
        vidx = res.tile([10, 1], F32, tag="vidx", name="vidx")
        nc.vector.tensor_copy(vidx[:], vidxI[:])

        ones = res.tile([128, 1], F32, tag="ones", name="ones")
        nc.vector.memset(ones[:], 1.0)

        # accumulators: rows 0..5 = S_term, S_utt, S_prop, m_term, m_utt, m_prop
        acc = res.tile([128, 6, NT], F32, tag="acc", name="acc")
        nc.vector.memset(acc[:], 0.0)
        nodes_a = res.tile([128, NT, 10], F32, tag="nodes_a", name="nodes_a")
        utt_a = res.tile([128, NT, MAX_LEN], I32, tag="utt_a", name="utt_a")
        prop_a = res.tile([128, NT, NUM_ITEMS], I32, tag="prop_a", name="prop_a")
        aterm_a = res.tile([128, NT], U8, tag="aterm_a", name="aterm_a")

        comb_acc = res.tile([E, BP], F32, tag="comb_acc", name="comb_acc")
        h_t = res.tile([E, BP], F32, tag="h_t", name="h_t")
        hcp = ctx.enter_context(tc.tile_pool(name="hcp", bufs=2))
        ohq_p = ctx.enter_context(tc.tile_pool(name="ohq_p", bufs=2))
        hlp = ctx.enter_context(tc.tile_pool(name="hlp", bufs=2))

        # ---- one-hot build ----
        def build_onehot(row, vocab):
            # step-0 onehot at base partition 0 (standalone x-side matmul)
            oh = ohp.tile([10, BP], BF16, tag="oh", name="oh")
            nc.gpsimd.dma_start(oh[:vocab, :], row_bcast_ap(tok_d, row, vocab, BP))
            nc.vector.tensor_scalar(oh[:vocab, :], oh[:vocab, :], vidx[:vocab, :],
                                    None, op0=OP.is_equal)
            return oh

        def build_onehot_hc(row, vocab, hc):
            # compute-engine partition ranges must be 32-aligned, so build the
            # onehot at base partition 0 and DMA it into rows E..E+vocab
            oh = ohp.tile([10, BP], BF16, tag="oh", name="oh")
            nc.gpsimd.dma_start(oh[:vocab, :], row_bcast_ap(tok_d, row, vocab, BP))
            nc.vector.tensor_scalar(oh[:vocab, :], oh[:vocab, :],
                                    vidx[:vocab, :], None, op0=OP.is_equal)
            nc.sync.dma_start(hc[E:E + vocab, :], oh[:vocab, :])

        # ---- LSTM step (feature-land), cols = chunk range ----
        def lstm_chunks(xw_t, vocab, wc_t, hw_t, b4_t, oh, h_prev, c_prev,
                        h_new, c_new, chunks, split_to=None, grp_sz=2):
            # h_prev: None (h=0, oh = standalone onehot tile) or (hc, hl) where
            # hc = [hh; onehot] combined bf16 tile; c_prev None => c=0
            xwh, xwl = xw_t
            wch, wcl = wc_t
            hwh, hwl = hw_t
            KC = E + vocab
            ch_list = list(chunks)
            for k0 in range(0, len(ch_list), grp_sz):
                grp = ch_list[k0:k0 + grp_sz]
                pst = {(g, k): psp.tile([E, CH], F32, tag="ps", name="ps")
                       for g in range(4) for k in grp}
                if h_prev is None:
                    passes = [(xwh[:vocab, :], oh[:vocab, :], True),
                              (xwl[:vocab, :], oh[:vocab, :], False)]
                else:
                    hc, hl = h_prev
                    passes = [(wch[:KC, :], hc[:KC, :], True),
                              (wcl[:KC, :], hc[:KC, :], False),
                              (hwh[:, :], hl[:, :], False),
                              (hwl[:, :], hl[:, :], False)]
                np_ = len(passes)
                # weight-reuse: same stationary operand streams both chunks
                for g in range(4):
                    gs = slice(g * E, (g + 1) * E)
                    for pi, (w, rhs_t, st) in enumerate(passes):
                        wv = w[:, gs]
                        for k in grp:
                            cols = slice(k * CH, (k + 1) * CH)
                            nc.tensor.matmul(pst[(g, k)][:], wv,
                                             rhs_t[:, cols],
                                             start=st, stop=pi == np_ - 1,
                                             skip_group_check=True)
                for k in grp:
                    cols = slice(k * CH, (k + 1) * CH)
                    ig = go.tile([E, CH], F32, tag="go", name="go")
                    nc.scalar.activation(ig[:], pst[(0, k)][:], AF.Sigmoid,
                                         bias=b4_t[:, 0:1])
                    og = go.tile([E, CH], F32, tag="go", name="go")
                    nc.scalar.activation(og[:], pst[(3, k)][:], AF.Sigmoid,
                                         bias=b4_t[:, 3:4])
                    if c_prev is None:
                        gg = go.tile([E, CH], F32, tag="go", name="go")
                        nc.scalar.activation(gg[:], pst[(2, k)][:], AF.Tanh,
                                             bias=b4_t[:, 2:3])
                        nc.vector.tensor_mul(c_new[:, cols], ig[:], gg[:])
                    else:
                        fg = go.tile([E, CH], F32, tag="go", name="go")
                        nc.scalar.activation(fg[:], pst[(1, k)][:], AF.Sigmoid,
                                             bias=b4_t[:, 1:2])
                        gg = go.tile([E, CH], F32, tag="go", name="go")
                        nc.scalar.activation(gg[:], pst[(2, k)][:], AF.Tanh,
                                             bias=b4_t[:, 2:3])
                        t1 = go.tile([E, CH], F32, tag="go", name="go")
                        nc.vector.tensor_mul(t1[:], ig[:], gg[:])
                        t2 = go.tile([E, CH], F32, tag="go", name="go")
                        nc.vector.tensor_mul(t2[:], fg[:], c_prev[:, cols])
                        nc.vector.tensor_add(c_new[:, cols], t1[:], t2[:])
                    th = go.tile([E, CH], F32, tag="go", name="go")
                    nc.scalar.activation(th[:], c_new[:, cols], AF.Tanh)
                    nc.vector.tensor_mul(h_new[:, cols], og[:], th[:])
                    if split_to is not None:
                        shc, shl = split_to
                        nc.scalar.copy(shc[:E, cols], h_new[:, cols])
                        nc.vector.tensor_tensor(shl[:, cols], h_new[:, cols],
                                                shc[:E, cols], op=OP.subtract)

        # ---- encoder ----
        def encoder(key, vocab, row0, steps):
            hsp = c_prev = None
            oh = build_onehot(row0, vocab)
            for s in range(steps):
                h_new = hp.tile([E, BP], F32, tag="h", name="h")
                c_new = cp.tile([E, BP], F32, tag="c", name="c")
                split_to = None
                if s + 1 < steps:
                    hc_next = hcp.tile([112, BP], BF16, tag="hc", name="hc")
                    build_onehot_hc(row0 + s + 1, vocab, hc_next)
                    split_to = (hc_next,
                                hlp.tile([E, BP], BF16, tag="hl", name="hl"))
                lstm_chunks(xw[key], vocab, wc[key], hw[key], b4[key], oh, hsp,
                            c_prev, h_new, c_new, range(NCH), split_to)
                hsp, c_prev = split_to, c_new
                oh = None
            return h_new

        def comb_add(enc_idx, h_enc):
            for k in range(NCH):
                cols = slice(k * CH, (k + 1) * CH)
                ps = psp.tile([128, CH], F32, tag="ps", name="ps")
                nc.tensor.matmul(ps[:E, :], cwT[enc_idx][:], h_enc[:, cols],
                                 start=True, stop=True)
                if enc_idx == 0:
                    nc.vector.tensor_copy(comb_acc[:, cols], ps[:E, :])
                else:
                    nc.vector.tensor_add(comb_acc[:, cols], comb_acc[:, cols],
                                         ps[:E, :])

        c_h = encoder("ctx", 6, 0, 6)
        comb_add(0, c_h)
        m_h = encoder("m", 10, 6, 6)
        comb_add(1, m_h)
        # ---- proposal encoder on the 216 distinct prev_proposal combos ----
        KP = 216
        poh = res.tile([6, 3 * KP], BF16, tag="poh", name="poh")
        nc.sync.dma_start(poh[:], poh_d.ap())
        vidx216 = []
        for half in range(2):
            vI = res.tile([108, 1], I32, tag=f"v216I{half}", name=f"v216I{half}")
            nc.gpsimd.iota(vI[:], pattern=[[0, 1]], base=108 * half,
                           channel_multiplier=1)
            vF = res.tile([108, 1], F32, tag=f"v216F{half}", name=f"v216F{half}")
            nc.vector.tensor_copy(vF[:], vI[:])
            vidx216.append(vF)
        o2p = ctx.enter_context(tc.tile_pool(name="o2p", bufs=2))

        def oh216_chunk(half, cols):
            t = o2p.tile([108, CH], BF16, tag="o2", name="o2")
            nc.gpsimd.dma_start(
                t[:], bass.AP(tensor=prop_id_d, offset=cols.start,
                              ap=[[0, 108], [1, CH]]))
            nc.vector.tensor_scalar(t[:], t[:], vidx216[half][:], None,
                                    op0=OP.is_equal)
            return t
        # mini LSTM over the 216 combos
        xwh_pr, xwl_pr = xw["pr"]
        wch_pr, wcl_pr = wc["pr"]
        hwh_pr, hwl_pr = hw["pr"]
        hc216 = hl216 = None
        h216 = c216 = None
        for s3 in range(3):
            hn = res.tile([E, KP], F32, tag=f"h216_{s3}", name=f"h216_{s3}")
            cn = res.tile([E, KP], F32, tag=f"c216_{s3}", name=f"c216_{s3}")
            pst = [psp.tile([E, KP], F32, tag="ps", name="ps") for _ in range(4)]
            if s3 == 0:
                passes = [(xwh_pr[:6, :], poh[:, 0:KP], True),
                          (xwl_pr[:6, :], poh[:, 0:KP], False)]
            else:
                passes = [(wch_pr[:106, :], hc216[:106, :], True),
                          (wcl_pr[:106, :], hc216[:106, :], False),
                          (hwh_pr[:, :], hl216[:, :], False),
                          (hwl_pr[:, :], hl216[:, :], False)]
            np3 = len(passes)
            for g in range(4):
                gs = slice(g * E, (g + 1) * E)
                for pi, (w, rhs_t, st) in enumerate(passes):
                    nc.tensor.matmul(pst[g][:], w[:, gs], rhs_t,
                                     start=st, stop=pi == np3 - 1,
                                     skip_group_check=True)
            b4p = b4["pr"]
            ig = go.tile([E, CH], F32, tag="go", name="go")[:, :KP]
            nc.scalar.activation(ig, pst[0][:], AF.Sigmoid, bias=b4p[:, 0:1])
            og = go.tile([E, CH], F32, tag="go", name="go")[:, :KP]
            nc.scalar.activation(og, pst[3][:], AF.Sigmoid, bias=b4p[:, 3:4])
            gg = go.tile([E, CH], F32, tag="go", name="go")[:, :KP]
            nc.scalar.activation(gg, pst[2][:], AF.Tanh, bias=b4p[:, 2:3])
            if s3 == 0:
                nc.vector.tensor_mul(cn[:], ig, gg)
            else:
                fg = go.tile([E, CH], F32, tag="go", name="go")[:, :KP]
                nc.scalar.activation(fg, pst[1][:], AF.Sigmoid, bias=b4p[:, 1:2])
                t1 = go.tile([E, CH], F32, tag="go", name="go")[:, :KP]
                nc.vector.tensor_mul(t1, ig, gg)
                t2 = go.tile([E, CH], F32, tag="go", name="go")[:, :KP]
                nc.vector.tensor_mul(t2, fg, c216[:])
                nc.vector.tensor_add(cn[:], t1, t2)
            th = go.tile([E, CH], F32, tag="go", name="go")[:, :KP]
            nc.scalar.activation(th, cn[:], AF.Tanh)
            nc.vector.tensor_mul(hn[:], og, th)
            if s3 < 2:
                hc216 = res.tile([112, KP], BF16, tag=f"hc216_{s3}",
                                 name=f"hc216_{s3}")
                nc.sync.dma_start(hc216[E:E + 6, :],
                                  poh[:, (s3 + 1) * KP:(s3 + 2) * KP])
                nc.scalar.copy(hc216[:E, :], hn[:])
                hl216 = res.tile([E, KP], BF16, tag=f"hl216_{s3}",
                                 name=f"hl216_{s3}")
                nc.vector.tensor_tensor(hl216[:], hn[:], hc216[:E, :],
                                        op=OP.subtract)
            h216, c216 = hn, cn
        # transpose h216 [100, 216] -> [216, 100] via DRAM round-trip
        rtT = drp.tile([KP * E], F32, tag="rtT", name="rtT")
        rb = rtT[:]
        nc.sync.dma_start(bass.AP(tensor=rb.tensor, offset=rb.offset,
                                  ap=[[1, E], [E, KP]]), h216[:])
        tabs = []
        for half in range(2):
            tf = res.tile([108, E], F32, tag=f"tabF{half}", name=f"tabF{half}")
            nc.sync.dma_start(tf[:], bass.AP(tensor=rb.tensor,
                                             offset=rb.offset + half * 108 * E,
                                             ap=[[E, 108], [1, E]]))
            tH = res.tile([108, E], BF16, tag=f"tabH{half}", name=f"tabH{half}")
            nc.scalar.copy(tH[:], tf[:])
            tL = res.tile([108, E], BF16, tag=f"tabL{half}", name=f"tabL{half}")
            nc.vector.tensor_tensor(tL[:], tf[:], tH[:], op=OP.subtract)
            tabs.append((tH, tL))
        # gather: p_h[:, b] = h216[:, prop_id[b]]
        p_h = hp.tile([E, BP], F32, tag="h", name="h")
        for k in range(NCH):
            cols = slice(k * CH, (k + 1) * CH)
            ohc2 = [oh216_chunk(half, cols) for half in range(2)]
            ps = psp.tile([128, CH], F32, tag="ps", name="ps")
            first = True
            for half in range(2):
                tH, tL = tabs[half]
                nc.tensor.matmul(ps[:E, :], tH[:], ohc2[half][:],
                                 start=first, stop=False, skip_group_check=True)
                first = False
                nc.tensor.matmul(ps[:E, :], tL[:], ohc2[half][:],
                                 start=False, stop=half == 1,
                                 skip_group_check=True)
            nc.scalar.copy(p_h[:, cols], ps[:E, :])
        comb_add(2, p_h)
        nc.scalar.activation(h_t[:], comb_acc[:], AF.Relu, bias=combb[:, 0:1])

        # ---- shared softmax/sample in head-land ----
        def softmax_sample(l2, C, nt, gum_ap, hsl, nodes_col, tok_slice,
                           ent_row, match_row):
            """l2: [128, nt, C] logits AP. Returns sampled index tile [128, nt]."""
            smax = hd.tile([128, nt], F32, tag="hs", name="hs")
            nc.vector.tensor_reduce(smax[:], l2, axis=AX.X, op=OP.max)
            sh = hd.tile([128, nt, 10], F32, tag="hd", name="hd")[:, :, :C]
            nc.vector.tensor_tensor(sh, l2, bc(smax[:], C), op=OP.subtract)
            ex = hd.tile([128, nt, 10], F32, tag="hd", name="hd")[:, :, :C]
            nc.scalar.activation(ex, sh, AF.Exp)
            S = hd.tile([128, nt], F32, tag="hs", name="hs")
            nc.vector.tensor_reduce(S[:], ex, axis=AX.X, op=OP.add)
            rec = hd.tile([128, nt], F32, tag="hs", name="hs")
            nc.vector.reciprocal(rec[:], S[:])
            probs = hd.tile([128, nt, 10], F32, tag="hd", name="hd")[:, :, :C]
            nc.vector.tensor_tensor(probs, ex, bc(rec[:], C), op=OP.mult)
            # entropy contribution sum_c (p+EPS)ln(p+EPS)
            q = hd.tile([128, nt, 10], F32, tag="hd", name="hd")[:, :, :C]
            nc.vector.tensor_single_scalar(q, probs, EPS, op=OP.add)
            lnq = hd.tile([128, nt, 10], F32, tag="hd", name="hd")[:, :, :C]
            nc.scalar.activation(lnq, q, AF.Ln)
            ql = hd.tile([128, nt, 10], F32, tag="hd", name="hd")[:, :, :C]
            nc.vector.tensor_mul(ql, q, lnq)
            ctr = hd.tile([128, nt], F32, tag="hs", name="hs")
            nc.vector.tensor_reduce(ctr[:], ql, axis=AX.X, op=OP.add)
            nc.vector.tensor_add(acc[:, ent_row, hsl], acc[:, ent_row, hsl],
                                 ctr[:])
            # categorical sample: argmax(sh + gumbel) (lnS shift cancels)
            v = hd.tile([128, nt, 10], F32, tag="hd", name="hd")[:, :, :C]
            nc.vector.tensor_tensor(v, sh, gum_ap, op=OP.add)
            vmax = hd.tile([128, nt], F32, tag="hs", name="hs")
            nc.vector.tensor_reduce(vmax[:], v, axis=AX.X, op=OP.max)
            veq = hd.tile([128, nt, 10], F32, tag="hd", name="hd")[:, :, :C]
            nc.vector.tensor_tensor(veq, v, bc(vmax[:], C), op=OP.is_equal)
            msk = hd.tile([128, nt, 10], F32, tag="hd", name="hd")[:, :, :C]
            nc.vector.tensor_mul(msk, veq, iotaC[:, :nt, :C])
            pen = hd.tile([128, nt, 10], F32, tag="hd", name="hd")[:, :, :C]
            nc.vector.tensor_scalar(pen, veq, -BIG, BIG, op0=OP.mult, op1=OP.add)
            nc.vector.tensor_add(msk, msk, pen)
            af = hd.tile([128, nt], F32, tag="hs", name="hs")
            nc.vector.tensor_reduce(af[:], msk, axis=AX.X, op=OP.min)
            nc.vector.tensor_copy(tok_slice, af[:])        # cast f32 -> i32
            # chosen prob -> nodes; greedy match via shifted-logit == 0
            aeq = hd.tile([128, nt, 10], F32, tag="hd", name="hd")[:, :, :C]
            nc.vector.tensor_tensor(aeq, iotaC[:, :nt, :C], bc(af[:], C),
                                    op=OP.is_equal)
            pa = hd.tile([128, nt, 10], F32, tag="hd", name="hd")[:, :, :C]
            nc.vector.tensor_mul(pa, probs, aeq)
            pa1 = hd.tile([128, nt], F32, tag="hs", name="hs")
            nc.vector.tensor_reduce(pa1[:], pa, axis=AX.X, op=OP.add)
            nc.scalar.activation(nodes_a[:, hsl, nodes_col], pa1[:], AF.Ln)
            ssel = hd.tile([128, nt, 10], F32, tag="hd", name="hd")[:, :, :C]
            nc.vector.tensor_mul(ssel, sh, aeq)
            ss1 = hd.tile([128, nt], F32, tag="hs", name="hs")
            nc.vector.tensor_reduce(ss1[:], ssel, axis=AX.X, op=OP.add)
            mt = hd.tile([128, nt], F32, tag="hs", name="hs")
            nc.vector.tensor_single_scalar(mt[:], ss1[:], 0.0, op=OP.is_equal)
            nc.vector.tensor_add(acc[:, match_row, hsl], acc[:, match_row, hsl],
                                 mt[:])
            return af

        # ---- term + proposal heads ----
        for half in range(2):
            hsl = slice(half * NH, (half + 1) * NH)
            ps19 = psp.tile([128, NH, 19], F32, tag="ps", name="ps")
            for t16 in range(NH):
                t = half * NH + t16
                nc.tensor.matmul(ps19[:, t16, :], h_t[:, t * 128:(t + 1) * 128],
                                 headW[:], start=True, stop=True)
            lg19 = hd19.tile([128, NH, 19], F32, tag="hd19", name="hd19")
            nc.vector.tensor_tensor(lg19[:], ps19[:], bcmid(headb[:], NH),
                                    op=OP.add)
            # term policy
            tp = hd.tile([128, NH], F32, tag="hs", name="hs")
            nc.scalar.activation(tp[:], lg19[:, :, 0], AF.Sigmoid)
            au = hd.tile([128, NH], F32, tag="hs", name="hs")
            nc.vector.tensor_tensor(au[:], ubern[:, hsl], tp[:], op=OP.is_lt)
            nc.vector.tensor_copy(aterm_a[:, hsl], au[:])   # cast -> u8
            rg = hd.tile([128, NH], F32, tag="hs", name="hs")
            nc.vector.tensor_single_scalar(rg[:], tp[:], 0.5, op=OP.is_ge)
            mt = hd.tile([128, NH], F32, tag="hs", name="hs")
            nc.vector.tensor_tensor(mt[:], rg[:], au[:], op=OP.is_equal)
            nc.vector.tensor_add(acc[:, 3, hsl], acc[:, 3, hsl], mt[:])
            # g = a*p + (1-a)*(1-p), exact fp32 mirror of the reference
            aup = hd.tile([128, NH], F32, tag="hs", name="hs")
            nc.vector.tensor_mul(aup[:], au[:], tp[:])
            nau = hd.tile([128, NH], F32, tag="hs", name="hs")
            nc.vector.tensor_scalar(nau[:], au[:], -1.0, 1.0, op0=OP.mult,
                                    op1=OP.add)
            ntp = hd.tile([128, NH], F32, tag="hs", name="hs")
            nc.vector.tensor_scalar(ntp[:], tp[:], -1.0, 1.0, op0=OP.mult,
                                    op1=OP.add)
            t2 = hd.tile([128, NH], F32, tag="hs", name="hs")
            nc.vector.tensor_mul(t2[:], nau[:], ntp[:])
            gt = hd.tile([128, NH], F32, tag="hs", name="hs")
            nc.vector.tensor_add(gt[:], aup[:], t2[:])
            nc.scalar.activation(nodes_a[:, hsl, 0], gt[:], AF.Ln)
            q = hd.tile([128, NH], F32, tag="hs", name="hs")
            nc.vector.tensor_single_scalar(q[:], tp[:], EPS, op=OP.add)
            lnq = hd.tile([128, NH], F32, tag="hs", name="hs")
            nc.scalar.activation(lnq[:], q[:], AF.Ln)
            ql = hd.tile([128, NH], F32, tag="hs", name="hs")
            nc.vector.tensor_mul(ql[:], q[:], lnq[:])
            nc.vector.tensor_add(acc[:, 0, hsl], acc[:, 0, hsl], ql[:])
            # proposal heads
            for i in range(NUM_ITEMS):
                softmax_sample(lg19[:, :, 1 + 6 * i:7 + 6 * i], 6, NH,
                               gump[:, i, hsl, :], hsl, 7 + i,
                               prop_a[:, hsl, i], 2, 5)

        # ---- utterance decode (autoregressive, pipelined in quarters) ----
        hc0 = hcp.tile([112, BP], BF16, tag="hc", name="hc")
        oh0 = ohp.tile([10, BP], BF16, tag="oh", name="oh")
        nc.vector.memset(oh0[:], 0.0)
        nc.vector.memset(oh0[0:1, :], 1.0)
        nc.sync.dma_start(hc0[E:E + 10, :], oh0[:])
        h_tl = hlp.tile([E, BP], BF16, tag="hl", name="hl")
        nc.scalar.copy(hc0[:E, :], h_t[:])
        nc.vector.tensor_tensor(h_tl[:], h_t[:], hc0[:E, :], op=OP.subtract)
        hsp = (hc0, h_tl)
        c_cur = None
        for s in range(MAX_LEN):
            rnp_s = rgp.tile([128, NT, 10], F32, tag="rnp", name="rnp")
            nc.sync.dma_start(rnp_s[:], rnp_d.ap()[:, s, :, :])
            gumu_s = rgp.tile([128, NT, 10], F32, tag="gumu", name="gumu")
            nc.sync.dma_start(gumu_s[:], gumu_d.ap()[:, s, :, :])
            h_new = hp.tile([E, BP], F32, tag="h", name="h")
            c_new = cp.tile([E, BP], F32, tag="c", name="c")
            split_to = None
            if s + 1 < MAX_LEN:
                split_to = (hcp.tile([112, BP], BF16, tag="hc", name="hc"),
                            hlp.tile([E, BP], BF16, tag="hl", name="hl"))
            for q in range(4):
                lstm_chunks(xw["up"], 10, wc["up"], hw["up"], b4["up"], None,
                            hsp, c_cur, h_new, c_new,
                            range(q * 2, q * 2 + 2), split_to, grp_sz=1)
                qsl = slice(q * NQT, (q + 1) * NQT)
                ps10 = psp.tile([128, NQT, 10], F32, tag="ps", name="ps")
                for tq in range(NQT):
                    t = q * NQT + tq
                    nc.tensor.matmul(ps10[:, tq, :],
                                     h_new[:, t * 128:(t + 1) * 128],
                                     uhW[:], start=True, stop=True)
                lg0 = hd.tile([128, NQT, 10], F32, tag="hd", name="hd")
                nc.vector.tensor_tensor(lg0[:], ps10[:], bcmid(uhb[:], NQT),
                                        op=OP.add)
                # noise blend: l2 = 0.9*l + 0.1*(lmin + (lmax-lmin)*rn)
                rmn = hd.tile([128, NQT], F32, tag="hs", name="hs")
                nc.vector.tensor_reduce(rmn[:], lg0[:], axis=AX.X, op=OP.min)
                rmx = hd.tile([128, NQT], F32, tag="hs", name="hs")
                nc.vector.tensor_reduce(rmx[:], lg0[:], axis=AX.X, op=OP.max)
                d = hd.tile([128, NQT], F32, tag="hs", name="hs")
                nc.vector.tensor_tensor(d[:], rmx[:], rmn[:], op=OP.subtract)
                n1 = hd.tile([128, NQT, 10], F32, tag="hd", name="hd")
                nc.vector.tensor_tensor(n1[:], rnp_s[:, qsl, :], bc(d[:], 10),
                                        op=OP.mult)
                noise = hd.tile([128, NQT, 10], F32, tag="hd", name="hd")
                nc.vector.tensor_tensor(noise[:], n1[:], bc(rmn[:], 10),
                                        op=OP.add)
                ns = hd.tile([128, NQT, 10], F32, tag="hd", name="hd")
                nc.vector.tensor_single_scalar(ns[:], noise[:], CORR, op=OP.mult)
                l2 = hd.tile([128, NQT, 10], F32, tag="hd", name="hd")
                nc.vector.scalar_tensor_tensor(l2[:], lg0[:], 1.0 - CORR, ns[:],
                                               op0=OP.mult, op1=OP.add)
                af = softmax_sample(l2[:], 10, NQT, gumu_s[:, qsl, :], qsl,
                                    1 + s, utt_a[:, qsl, s], 1, 4)
                if s + 1 < MAX_LEN:
                    hc_next = split_to[0]
                    rt = drp.tile([BQ], F32, tag="rt", name="rt")
                    rbase = rt[:]
                    nc.sync.dma_start(
                        bass.AP(tensor=rbase.tensor, offset=rbase.offset,
                                ap=[[1, 128], [128, NQT]]), af[:])
                    cols = slice(q * BQ, (q + 1) * BQ)
                    ohq = ohq_p.tile([10, BQ], BF16, tag="ohq", name="ohq")
                    nc.gpsimd.dma_start(
                        ohq[:],
                        bass.AP(tensor=rbase.tensor, offset=rbase.offset,
                                ap=[[0, 10], [1, BQ]]))
                    nc.vector.tensor_scalar(ohq[:], ohq[:], vidx[:], None,
                                            op0=OP.is_equal)
                    nc.sync.dma_start(hc_next[E:E + 10, cols], ohq[:])
            c_cur = c_new
            hsp = split_to

        # ---- final scalars ----
        red6 = hd.tile([128, 6], F32, tag="red6", name="red6")
        nc.vector.tensor_reduce(red6[:], acc[:], axis=AX.X, op=OP.add)
        ps_s = psp.tile([1, 6], F32, tag="ps", name="ps")
        nc.tensor.matmul(ps_s[:], ones[:], red6[:], start=True, stop=True)
        ssb = hd.tile([1, 6], F32, tag="ssb", name="ssb")
        nc.vector.tensor_copy(ssb[:], ps_s[:])
        nc.sync.dma_start(scal_o.ap(), ssb[:])

        # ---- output DMAs (head-land -> [BP, k] row-major) ----
        def out_ap(handle, k):
            return bass.AP(tensor=handle, offset=0,
                           ap=[[k, 128], [128 * k, NT], [1, k]])

        nc.sync.dma_start(out_ap(nodes_o, 10), nodes_a[:])
        nc.sync.dma_start(out_ap(utt_o, MAX_LEN), utt_a[:])
        nc.sync.dma_start(out_ap(prop_o, NUM_ITEMS), prop_a[:])
        nc.sync.dma_start(
            bass.AP(tensor=aterm_o, offset=0, ap=[[1, 128], [128, NT]]),
            aterm_a[:])

    _split_excess_sync(nc)
    return nc


# ---------------------------------------------------------------- host prep
def _rng_draws():
    if "rng" in _CACHE:
        return _CACHE["rng"]
    import jax
    import jax.numpy as jnp
    cpu = jax.devices('cpu')[0]
    with jax.default_device(cpu):
        rng = jax.random.key(1234)
        u = np.asarray(jax.random.uniform(jax.random.fold_in(rng, 0),
                                          (BTOT, 1), jnp.float32))
        raws = [np.asarray(jax.random.normal(jax.random.fold_in(rng, 100 + i),
                                             (BTOT, 10), jnp.float32))
                for i in range(MAX_LEN)]
        gu = [np.asarray(jax.random.gumbel(jax.random.fold_in(rng, 200 + i),
                                           (BTOT, 10), jnp.float32))
              for i in range(MAX_LEN)]
        gp = [np.asarray(jax.random.gumbel(jax.random.fold_in(rng, 300 + i),
                                           (BTOT, 6), jnp.float32))
              for i in range(NUM_ITEMS)]
    rn = []
    for r in raws:
        nmin = r.min(axis=1, keepdims=True)
        nmax = r.max(axis=1, keepdims=True)
        rn.append(((r - nmin) / (nmax - nmin)).astype(np.float32))
    _CACHE["rng"] = (u, rn, gu, gp)
    return _CACHE["rng"]


def _hl(x):
    """[BP, k] (or [BP]) -> head-land [128, NT, k] / [128, NT]."""
    if x.ndim == 1:
        return np.ascontiguousarray(x.reshape(NT, 128).T)
    return np.ascontiguousarray(
        x.reshape(NT, 128, x.shape[1]).transpose(1, 0, 2))


def _bf16_split(x):
    import ml_dtypes
    bf16 = ml_dtypes.bfloat16
    xh = x.astype(bf16)
    xl = (x - xh.astype(np.float32)).astype(bf16)
    return np.ascontiguousarray(xh), np.ascontiguousarray(xl)


def _prep_in_maps(pool, utility, m_prev, prev_proposal, params):
    f32 = np.float32
    P = {k: np.asarray(v, f32) for k, v in params.items()}
    toks = np.stack([np.asarray(t, np.int64)[:, j].astype(f32)
                     for t, jr in [(pool, range(3)), (utility, range(3)),
                                   (m_prev, range(6)), (prev_proposal, range(3))]
                     for j in jr])  # [15, BTOT]

    wmap = {
        "b4_ctx": np.ascontiguousarray((P["ctx_bih"] + P["ctx_bhh"]).reshape(4, E).T),
        "b4_m": np.ascontiguousarray((P["utt_bih"] + P["utt_bhh"]).reshape(4, E).T),
        "b4_pr": np.ascontiguousarray((P["prop_bih"] + P["prop_bhh"]).reshape(4, E).T),
        "b4_up": np.ascontiguousarray((P["up_bih"] + P["up_bhh"]).reshape(4, E).T),
        "cwT": np.ascontiguousarray(P["comb_W"].T),
        "combb": P["comb_b"].reshape(E, 1),
        "headW": np.concatenate([P["term_W"].T] +
                                [P["pp_W"][i].T for i in range(NUM_ITEMS)], axis=1),
        "headb": np.broadcast_to(
            np.concatenate([P["term_b"]] + [P["pp_b"][i] for i in range(NUM_ITEMS)]),
            (128, 19)).copy(),
        "uhW": np.ascontiguousarray(P["up_h1_W"].T),
        "uhb": np.broadcast_to(P["up_h1_b"], (128, 10)).copy(),
    }
    wmap = {k: np.ascontiguousarray(v, dtype=f32) for k, v in wmap.items()}
    for key, xwm, hwm in [
            ("ctx", P["emb_ctx"] @ P["ctx_Wih"].T, P["ctx_Whh"].T),
            ("m", P["emb_utt"] @ P["utt_Wih"].T, P["utt_Whh"].T),
            ("pr", P["emb_ctx"] @ P["prop_Wih"].T, P["prop_Whh"].T),
            ("up", P["up_emb"] @ P["up_Wih"].T, P["up_Whh"].T)]:
        xh, xl = _bf16_split(np.ascontiguousarray(xwm, dtype=f32))
        hh, hl = _bf16_split(np.ascontiguousarray(hwm, dtype=f32))
        wmap[f"xwh_{key}"], wmap[f"xwl_{key}"] = xh, xl
        wmap[f"hwh_{key}"], wmap[f"hwl_{key}"] = hh, hl
        wmap[f"wch_{key}"] = np.concatenate([hh, xh], axis=0)
        wmap[f"wcl_{key}"] = np.concatenate([hl, xl], axis=0)

    import ml_dtypes
    pp_i = np.asarray(prev_proposal, np.int64)
    prop_id_full = (36 * pp_i[:, 0] + 6 * pp_i[:, 1] + pp_i[:, 2]).astype(f32)
    combos = np.arange(216)
    digits = np.stack([combos // 36, (combos // 6) % 6, combos % 6])  # [3, 216]
    poh = np.zeros((6, 3 * 216), np.float32)
    for t in range(3):
        poh[:, t * 216:(t + 1) * 216] = (digits[t][None, :] ==
                                         np.arange(6)[:, None])
    poh = poh.astype(ml_dtypes.bfloat16)

    u, rn, gu, gp = _rng_draws()
    in_maps = []
    for c in range(NCORES):
        sl = slice(c * BP, (c + 1) * BP)
        m = dict(wmap)
        m["tok"] = np.ascontiguousarray(toks[:, sl])
        m["prop_id"] = np.ascontiguousarray(prop_id_full[sl])
        m["poh"] = poh
        m["ubern"] = _hl(u[sl, 0])
        m["rnp"] = np.ascontiguousarray(
            np.stack([_hl(rn[i][sl]) for i in range(MAX_LEN)], axis=1))
        m["gumu"] = np.ascontiguousarray(
            np.stack([_hl(gu[i][sl]) for i in range(MAX_LEN)], axis=1))
        m["gump"] = np.ascontiguousarray(
            np.stack([_hl(gp[i][sl]) for i in range(NUM_ITEMS)], axis=1))
        in_maps.append(m)
    return in_maps


def _get_nc():
    if "nc" not in _CACHE:
        _CACHE["nc"] = _build_program()
    return _CACHE["nc"]


def _run(in_maps, trace=False, trace_kwargs=None):
    from concourse.bass_utils import run_bass_kernel_spmd
    nc = _get_nc()
    kw = {}
    if trace:
        kw["trace"] = True
        if trace_kwargs:
            kw["trace_kwargs"] = trace_kwargs
    return run_bass_kernel_spmd(nc, in_maps, core_ids=list(range(NCORES)), **kw)


def _assemble(results):
    nodes = np.concatenate([r["nodes_o"] for r in results], axis=0)
    a_term = np.concatenate([r["aterm_o"] for r in results])[:, None]
    utterance = np.concatenate([r["utt_o"] for r in results], axis=0)
    proposal = np.concatenate([r["prop_o"] for r in results], axis=0)
    scal = np.stack([r["scal_o"][0] for r in results]).astype(np.float64)
    s_term, s_utt, s_prop, m_term, m_utt, m_prop = scal.sum(axis=0)
    entropy_loss = np.float32(0.05 * s_term + 0.001 * s_utt + 0.05 * s_prop)
    return (nodes.astype(np.float32), a_term.astype(np.uint8),
            utterance.astype(np.int32), proposal.astype(np.int32),
            entropy_loss, np.int32(round(m_term)), np.int32(round(m_utt)),
            MAX_LEN * BTOT, np.int32(round(m_prop)), NUM_ITEMS * BTOT)


def kernel(pool, utility, m_prev, prev_proposal, params):
    in_maps = _prep_in_maps(pool, utility, m_prev, prev_proposal, params)
    res = _run(in_maps, trace=False)
    return _assemble(res.results)
